# revision 27
# baseline (speedup 1.0000x reference)
"""Trainium2 Bass kernel for nn_CAPMemory (camera-aware proxy memory loss).

Strategy (8 NeuronCores, SPMD, no collectives):
  - Shard the 64000x256 proxy table over labels: core k owns labels
    [1000k, 1000(k+1)), all 8 cameras.  On the host the shard is laid out
    CAMERA-MAJOR with each camera block padded 1000 -> 1024 columns
    (pad centers = 0 vectors): col c*1024 + l holds proxy (label l, cam c).
    This aligns camera blocks with PSUM banks and makes the intra-camera
    softmax read contiguous.
  - Feats are normalized and transposed on the host; the device runs a pure
    pipeline: DMA -> matmul (f32r, two 128-contraction halves accumulated in
    PSUM) -> per-1024-column-unit drain -> small outputs.
  - Each (row-tile, camera-block) unit [128 x 1024] in PSUM is drained by
    exactly one engine:
      direct unit: DVE MAX8 straight from PSUM -> top-8 values (f32)
      exp unit   : ACT Exp(scale=20) from PSUM -> bf16 exp values in SBUF
                   (+ accumulated per-camera exp-sum for the intra loss)
      window unit: ACT Copy from PSUM -> bf16 sims in SBUF
    bf16 units then go through a DVE pairwise-max tree (2x perf mode) to
    8-wide window maxes [128 x 125] shipped to the host (exp-domain values
    for exp units; the host takes log/20).
  - Host merge: intra logsumexp = log(sum_k srow_k); inter top-50 hard
    negatives merged from per-block top-8s and window maxes, positives
    removed by eps value matching; positives themselves recomputed exactly
    on host in f64.
  - Certificate on direct blocks (8th value <= merged t50) triggers exact
    per-row host recomputation; window blocks are statistically covered
    (window collisions lose at most one near-cutoff negative, effect on the
    loss ~1e-4 relative, validated offline against the reference).
"""

import sys
import functools

sys.path.insert(0, "/opt/trn_rl_repo")

import numpy as np
import ml_dtypes

from concourse import bacc, mybir
from concourse.tile import TileContext

F32 = mybir.dt.float32
BF16 = mybir.dt.bfloat16

N = 512          # batch
D = 256          # feature dim
L = 8000         # labels
C = 8            # cameras
NCORES = 8
RT = 4           # row tiles of 128
L_LOCAL = 1000   # labels per core
BPAD = 1024      # padded camera-block width
P_PAD = C * BPAD  # 8192 padded columns per core
INV_T = 20.0     # 1 / temperature
K = 50           # hard negatives
LW = 0.5         # inter-cam loss weight
NW = L_LOCAL // 8  # 125 8-wide windows per camera block

# total units drained by DVE MAX8 straight from PSUM (top-8 candidates);
# the rest are drained by ACT (exp or copy) and window-max-treed on DVE.
# ACT and DVE are the only engines that can read PSUM, so this splits the
# [512 x 8192] drain across both.
DIRECT_TOTAL = 10

# matmul operand dtype: float8e4 with the DoubleRow perf mode -- both
# 128-contraction halves fold into ONE matmul at 0.5 cyc/output-col (4x the
# bf16/f32r PE rate) and the centers DMA shrinks to 2.1 MB.  Quantization
# noise on the cosine sims is ~3e-3 RMS; end-to-end loss error validated at
# ~6e-4 relative (tolerance 2e-2).  Positives are recomputed exactly on the
# host, and the eps value-matching margins below absorb the noise.
MM_DT = mybir.dt.float8e4
MM_NP = ml_dtypes.float8_e4m3


def _pair_order(sizes):
    """Order cameras big+small so most 128-row tiles span only ~2 cameras."""
    desc = np.argsort(-np.asarray(sizes), kind="stable")
    big, small = desc[: C // 2], desc[C // 2 :][::-1]
    order = []
    for b, s in zip(big, small):
        order += [int(b), int(s)]
    return order


def _unit_plan(tile_cams):
    """Static drain plan: units in b-major order, kind per unit, tree pairs.

    Returns (units, kind, pairs) where pairs maps a pair id to the list of
    its member units (1 or 2, same row-tile) and each tree unit knows its
    (pair id, slot).
    """
    import itertools

    units = [(rt, b) for b in range(C) for rt in range(RT)]
    kind = {}
    # distribute DIRECT_TOTAL direct units across row tiles (more directs to
    # tiles with fewer exp units), choosing blocks so the remaining tree
    # blocks pair up adjacently (single-instruction tree last rounds)
    nexp = [len(tile_cams[rt]) for rt in range(RT)]
    d_rt = [0] * RT
    for _ in range(DIRECT_TOTAL):
        # give the next direct to the tile with most unassigned non-exp blocks
        loads = [(C - nexp[rt] - d_rt[rt], -rt) for rt in range(RT)]
        rt = -max(loads)[1]
        d_rt[rt] += 1

    def adj_score(tree_bs):
        s = 0
        bs = sorted(tree_bs)
        for i in range(0, len(bs) - 1, 2):
            if bs[i + 1] == bs[i] + 1:
                s += 1
        return s

    for rt in range(RT):
        nonexp = [b for b in range(C) if b not in tile_cams[rt]]
        best = None
        for directs in itertools.combinations(nonexp, d_rt[rt]):
            rest = [b for b in range(C) if b not in directs]
            sc = adj_score([b for b in rest])
            if best is None or sc > best[0]:
                best = (sc, directs)
        directs = set(best[1])
        for b in range(C):
            if b in tile_cams[rt]:
                kind[(rt, b)] = "exp"
            elif b in directs:
                kind[(rt, b)] = "direct"
            else:
                kind[(rt, b)] = "win"
    # pair tree units (exp+win) within each row tile in b order
    pair_of = {}
    pairs = []
    for rt in range(RT):
        tus = [(rt, b) for b in range(C) if kind[(rt, b)] != "direct"]
        for i in range(0, len(tus), 2):
            members = tus[i : i + 2]
            pid = len(pairs)
            pairs.append(members)
            for s, u in enumerate(members):
                pair_of[u] = (pid, s)
    return units, kind, pairs, pair_of


@functools.lru_cache(maxsize=8)
def _build_program(tile_cams, repeats=1):
    nc = bacc.Bacc(None, target_bir_lowering=False, num_swdge_queues=4)

    cenTd = nc.dram_tensor("cenT", [2, 128, P_PAD], MM_DT, kind="ExternalInput")
    fTd = nc.dram_tensor("fT", [128, RT, 2, 128], MM_DT, kind="ExternalInput")
    candd = nc.dram_tensor("cand", [RT, 128, C * 8], F32, kind="ExternalOutput")
    srowd = nc.dram_tensor("srow", [RT, 128, C], F32, kind="ExternalOutput")
    wmaxd = nc.dram_tensor("wmax", [RT, 128, C, NW], BF16, kind="ExternalOutput")

    with TileContext(nc) as tc:
        with (
            tc.tile_pool(name="cen", bufs=2) as cenp,
            tc.tile_pool(name="ftp", bufs=2) as ftp,
            tc.tile_pool(name="scrp", bufs=5) as scrp,
            tc.tile_pool(name="treep", bufs=3) as treep,
            tc.tile_pool(name="outp", bufs=2) as outp,
            tc.tile_pool(name="psum", bufs=4, space="PSUM") as psump,
        ):
            for _rep in range(repeats):
                _kernel_body(nc, tc, cenp, ftp, scrp, treep, outp, psump,
                             cenTd, fTd, candd, srowd, wmaxd, tile_cams)

    nc.compile()
    return nc


def _kernel_body(nc, tc, cenp, ftp, scrp, treep, outp, psump,
                 cenTd, fTd, candd, srowd, wmaxd, tile_cams):
    ActF = mybir.ActivationFunctionType

    units, kind, pairs, pair_of = _unit_plan(tile_cams)

    # ---- input DMA: fT first (matmuls need it immediately), then centers in
    # (h, block) granularity so early units unblock fast; alternate issuing
    # engines to spread descriptor generation across queues
    # inputs on SP only (so the next repeat's loads never queue behind this
    # repeat's output descriptors); outputs on gpsimd only
    fT_sb = ftp.tile([128, RT, 2, 128], MM_DT, name="fT_sb")
    nc.sync.dma_start(out=fT_sb[:, :, :, :], in_=fTd[:, :, :, :])
    cen_sb = cenp.tile([128, 2, P_PAD], MM_DT, name="cen_sb")
    for b in range(C):
        sl = slice(b * BPAD, (b + 1) * BPAD)
        for h in range(2):
            nc.sync.dma_start(out=cen_sb[:, h, sl], in_=cenTd[h, :, sl])

    cand_sb = [outp.tile([128, C * 8], F32, name=f"cand{rt}", bufs=1)
               for rt in range(RT)]
    s_t = [outp.tile([128, C], F32, name=f"st{rt}", bufs=1)
           for rt in range(RT)]
    wm_rt = [outp.tile([128, C, NW], BF16, name=f"wm{rt}", bufs=1)
             for rt in range(RT)]

    # pair state: scr tiles allocated lazily, members drain at different times
    pair_scr = [None] * len(pairs)
    pair_filled = [0] * len(pairs)
    # how many direct/exp/tree units remain per rt (to time the output DMAs)
    left_direct = [sum(1 for b in range(C) if kind[(rt, b)] == "direct")
                   for rt in range(RT)]
    left_exp = [len(tile_cams[rt]) for rt in range(RT)]
    left_tree = [sum(1 for b in range(C) if kind[(rt, b)] != "direct")
                 for rt in range(RT)]

    for ui, (rt, b) in enumerate(units):
        ps = psump.tile([128, BPAD], F32, name="ps")
        c0 = b * BPAD
        for j in range(2):
            nc.tensor.matmul(
                ps[:, j * 512 : (j + 1) * 512],
                fT_sb[:, rt, :, :],
                cen_sb[:, :, c0 + j * 512 : c0 + (j + 1) * 512],
                start=True, stop=True,
                perf_mode=mybir.MatmulPerfMode.DoubleRow,
            )
        k = kind[(rt, b)]
        if k == "direct":
            nc.vector.max(cand_sb[rt][:, b * 8 : b * 8 + 8], ps[:, 0:L_LOCAL])
            left_direct[rt] -= 1
            if left_direct[rt] == 0:
                nc.gpsimd.dma_start(out=candd[rt], in_=cand_sb[rt][:, :])
            continue
        pid, slot = pair_of[(rt, b)]
        npair = len(pairs[pid])
        if pair_scr[pid] is None:
            pair_scr[pid] = scrp.tile([128, npair, L_LOCAL], BF16, name="scr")
        scr = pair_scr[pid]
        if k == "exp":
            idx = tile_cams[rt].index(b)
            nc.scalar.activation(
                scr[:, slot, :], ps[:, 0:L_LOCAL], ActF.Exp,
                scale=INV_T, accum_out=s_t[rt][:, idx : idx + 1],
            )
            left_exp[rt] -= 1
            if left_exp[rt] == 0:
                nc.gpsimd.dma_start(out=srowd[rt], in_=s_t[rt][:, :])
        else:
            nc.scalar.copy(scr[:, slot, :], ps[:, 0:L_LOCAL])
        pair_filled[pid] += 1
        if pair_filled[pid] == npair:
            # bf16 pairwise-max tree: [np,125,8] -> per-block window maxes;
            # when the pair's blocks are adjacent the last round writes one
            # contiguous wm_rt slice, otherwise one instruction per member
            v = scr.rearrange("p np (nw w) -> p np nw w", w=8)
            t1 = treep.tile([128, npair, NW, 4], BF16, name="t1")
            t2 = treep.tile([128, npair, NW, 2], BF16, name="t2")
            nc.vector.tensor_max(t1[:, :, :, :], v[:, :, :, 0:4],
                                 v[:, :, :, 4:8])
            nc.vector.tensor_max(t2[:, :, :, :], t1[:, :, :, 0:2],
                                 t1[:, :, :, 2:4])
            dst = wm_rt[rt].rearrange("p c (nw one) -> p c nw one", one=1)
            bs = [u[1] for u in pairs[pid]]
            if npair == 2 and bs[1] == bs[0] + 1:
                nc.vector.tensor_max(dst[:, bs[0] : bs[0] + 2, :, :],
                                     t2[:, :, :, 0:1], t2[:, :, :, 1:2])
            else:
                for s, bu in enumerate(bs):
                    nc.vector.tensor_max(dst[:, bu : bu + 1, :, :],
                                         t2[:, s : s + 1, :, 0:1],
                                         t2[:, s : s + 1, :, 1:2])
            left_tree[rt] -= npair
            if left_tree[rt] == 0:
                nc.gpsimd.dma_start(out=wmaxd[rt], in_=wm_rt[rt][:, :, :])


class _Runner:
    """Sharded 8-core executor for a built Bass program.

    Builds the jax.jit(shard_map(bass_exec)) executable once (the walrus/NEFF
    compile happens inside the first call) and reuses it for every subsequent
    execution, keeping large inputs device-resident.
    """

    def __init__(self, nc, n_cores=NCORES):
        import jax
        from jax.sharding import Mesh, PartitionSpec, NamedSharding
        from jax.experimental.shard_map import shard_map
        from concourse import bass2jax

        self.jax = jax
        self.nc = nc
        self.n_cores = n_cores
        bass2jax.install_neuronx_cc_hook()
        partition_name = (
            nc.partition_id_tensor.name if nc.partition_id_tensor else None
        )
        in_names, out_names, out_avals = [], [], []
        for alloc in nc.m.functions[0].allocations:
            if not isinstance(alloc, mybir.MemoryLocationSet):
                continue
            name = alloc.memorylocations[0].name
            if alloc.kind == "ExternalInput":
                if name != partition_name:
                    in_names.append(name)
            elif alloc.kind == "ExternalOutput":
                out_names.append(name)
                out_avals.append(
                    jax.core.ShapedArray(
                        tuple(alloc.tensor_shape), mybir.dt.np(alloc.dtype)
                    )
                )
        self.in_names, self.out_names, self.out_avals = in_names, out_names, out_avals
        n_params, n_outs = len(in_names), len(out_avals)
        all_in_names = list(in_names) + list(out_names)
        if partition_name is not None:
            all_in_names.append(partition_name)

        def _body(*args):
            operands = list(args)
            if partition_name is not None:
                operands.append(bass2jax.partition_id_tensor())
            return tuple(
                bass2jax._bass_exec_p.bind(
                    *operands,
                    out_avals=tuple(out_avals),
                    in_names=tuple(all_in_names),
                    out_names=tuple(out_names),
                    lowering_input_output_aliases=(),
                    sim_require_finite=True,
                    sim_require_nnan=True,
                    nc=nc,
                )
            )

        devices = jax.devices()[:n_cores]
        self.mesh = Mesh(np.asarray(devices), ("core",))
        self.sh = NamedSharding(self.mesh, PartitionSpec("core"))
        self.fn = jax.jit(
            shard_map(
                _body,
                mesh=self.mesh,
                in_specs=(PartitionSpec("core"),) * (n_params + n_outs),
                out_specs=(PartitionSpec("core"),) * n_outs,
                check_rep=False,
            ),
            donate_argnums=tuple(range(n_params, n_params + n_outs)),
            keep_unused=True,
        )
        self._zero_shapes = [
            ((n_cores * a.shape[0], *a.shape[1:]), a.dtype) for a in out_avals
        ]

    def put_inputs(self, in_maps):
        self.dev_in = [
            self.jax.device_put(
                np.concatenate([np.asarray(m[name]) for m in in_maps], axis=0),
                self.sh,
            )
            for name in self.in_names
        ]

    def _zeros(self):
        return [
            self.jax.device_put(np.zeros(s, d), self.sh)
            for s, d in self._zero_shapes
        ]

    def execute(self):
        outs = self.fn(*self.dev_in, *self._zeros())
        self.jax.block_until_ready(outs)
        return self.unpack(outs)

    def unpack(self, outs):
        return [
            {
                name: np.asarray(outs[i]).reshape(
                    self.n_cores, *self.out_avals[i].shape
                )[c]
                for i, name in enumerate(self.out_names)
            }
            for c in range(self.n_cores)
        ]


_RUNNERS = {}
_LAST_FALLBACKS = 0
_FORCE_FALLBACK = False  # test hook: exercise the exact host fallback path


def _get_runner(nc):
    r = _RUNNERS.get(id(nc))
    if r is None:
        r = _Runner(nc)
        _RUNNERS[id(nc)] = r
    return r


def _make_in_maps(cenT_shards, feats_p):
    # feats_p is the permuted, L2-normalized batch; device wants the
    # transposed layout [q, rt, h, r] with q the contraction partition
    fT = np.ascontiguousarray(
        feats_p.reshape(RT, 128, 2, 128).transpose(3, 0, 2, 1), dtype=MM_NP
    )
    return [
        {"cenT": np.ascontiguousarray(cenT_shards[k], dtype=MM_NP), "fT": fT}
        for k in range(NCORES)
    ]


def _host_finish(results, feats_p, labels_p, cams_p, centers, tile_cams):
    units, kind, pairs, pair_of = _unit_plan(tile_cams)
    rows = np.arange(N)
    rt_of = rows // 128
    p_of = rows % 128

    # ---- intra: sum over cores of per-camera exp sums ----
    slot = np.zeros(N, dtype=np.int64)
    for rt in range(RT):
        for idx, cam in enumerate(tile_cams[rt]):
            sel = slice(128 * rt, 128 * (rt + 1))
            slot[sel] = np.where(cams_p[sel] == cam, idx, slot[sel])
    s_k = np.stack(
        [
            results[k]["srow"].reshape(RT, 128, C)[rt_of, p_of, slot]
            for k in range(NCORES)
        ]
    ).astype(np.float64)  # [8, 512]: sum_l exp(20 * cos sims) per core

    fn = feats_p.astype(np.float64)
    fn = fn / np.linalg.norm(fn, axis=1, keepdims=True)
    cen = centers.astype(np.float64)
    gidx = labels_p[:, None] * C + np.arange(C)[None, :]        # [512, 8]
    pos = np.einsum("rcd,rd->rc", cen[gidx], fn)                # [512, 8] f64

    lse_intra = np.log(s_k.sum(axis=0))
    v = pos[rows, cams_p]
    loss_intra_i = lse_intra - INV_T * v

    # ---- inter: merge candidates ----
    # direct blocks contribute their top-8 values; tree blocks their 125
    # window maxes (exp blocks in exp domain: s = log(w)/20).
    cand = np.stack([results[k]["cand"] for k in range(NCORES)])  # [8,RT,128,64]
    wmax = np.stack(
        [results[k]["wmax"].astype(np.float32) for k in range(NCORES)]
    )  # [8,RT,128,C,NW]

    # convert exp-domain window maxes back to sims domain; mask the window
    # slots of direct blocks (their wm_rt slices are stale pool memory)
    for rt in range(RT):
        for b in range(C):
            kd = kind[(rt, b)]
            if kd == "exp":
                w = wmax[:, rt, :, b, :]
                wmax[:, rt, :, b, :] = np.log(np.maximum(w, 1e-30)) / INV_T
            elif kd == "direct":
                wmax[:, rt, :, b, :] = -np.inf

    wspan = C * NW
    cspan = C * 8
    span = wspan + cspan
    CR = np.empty((N, NCORES * span), dtype=np.float64)
    # window part: CR[i, k*span + b*NW + w]
    CR[:, : NCORES * wspan].reshape(N, NCORES, wspan)[:] = (
        wmax[:, rt_of, p_of, :, :].transpose(1, 0, 2, 3).reshape(N, NCORES, wspan)
    )
    # direct part: CR[i, NCORES*wspan + k*cspan + b*8 + j], -inf for non-direct
    cpart = cand[:, rt_of, p_of, :].transpose(1, 0, 2)          # [512, 8, 64]
    dmask = np.zeros((N, cspan), dtype=bool)
    for rt in range(RT):
        rsel = rt_of == rt
        for b in range(C):
            if kind[(rt, b)] == "direct":
                dmask[rsel, b * 8 : b * 8 + 8] = True
    CR[:, NCORES * wspan :] = np.where(
        dmask[:, None, :], cpart, -np.inf
    ).reshape(N, NCORES * cspan)

    # ---- remove positives by eps value matching ----
    owner = labels_p // L_LOCAL
    lloc = labels_p % L_LOCAL
    win = lloc // 8
    EPS = 1.5e-2
    for i in rows:
        rt = rt_of[i]
        k0 = owner[i]
        for c in range(C):
            if kind[(rt, c)] == "direct":
                idxs = np.arange(NCORES * wspan + k0 * cspan + c * 8,
                                 NCORES * wspan + k0 * cspan + c * 8 + 8)
                vals = CR[i, idxs]
                j = int(np.argmin(np.abs(vals - pos[i, c])))
                if abs(vals[j] - pos[i, c]) < EPS:
                    CR[i, idxs[j]] = -np.inf
            else:
                jj = k0 * wspan + c * NW + win[i]
                if abs(CR[i, jj] - pos[i, c]) < EPS:
                    CR[i, jj] = -np.inf

    part = np.partition(CR, CR.shape[1] - K, axis=1)[:, -K:]
    t50 = part.min(axis=1)

    # ---- certificate on direct blocks: 8th value must be <= t50 ----
    if _FORCE_FALLBACK:
        bad = rows
    else:
        worst = np.full(N, -np.inf)
        for rt in range(RT):
            rsel = rt_of == rt
            for b in range(C):
                if kind[(rt, b)] != "direct":
                    continue
                worst[rsel] = np.maximum(
                    worst[rsel],
                    cand[:, rt, :, b * 8 + 7].max(axis=0)[p_of[rsel]],
                )
        bad = np.where(worst > t50)[0]
    global _LAST_FALLBACKS
    _LAST_FALLBACKS = len(bad)
    for i in bad:
        sims_row = cen @ fn[i]                                  # [64000] exact
        sims_row[C * labels_p[i] : C * labels_p[i] + C] = -np.inf
        part[i] = np.sort(sims_row)[-K:]

    z = np.concatenate([pos, part], axis=1) * INV_T             # [512, 58]
    mz = z.max(axis=1)
    lse_inter = np.log(np.exp(z - mz[:, None]).sum(axis=1)) + mz
    loss_inter_i = lse_inter - INV_T * pos.mean(axis=1)

    # ---- per-camera means, summed ----
    cnt = np.bincount(cams_p, minlength=C).astype(np.float64)
    s_intra = np.bincount(cams_p, weights=loss_intra_i, minlength=C)
    s_inter = np.bincount(cams_p, weights=loss_inter_i, minlength=C)
    safe = np.maximum(cnt, 1.0)
    li = np.sum(np.where(cnt > 0, s_intra / safe, 0.0))
    le = LW * np.sum(np.where(cnt > 0, s_inter / safe, 0.0))
    return np.array([li, le], dtype=np.float32)


def _prepare(feats, indexes, label_table, cam_table, centers):
    feats = np.asarray(feats, dtype=np.float32)
    indexes = np.asarray(indexes)
    label_table = np.asarray(label_table)
    cam_table = np.asarray(cam_table)
    centers = np.asarray(centers, dtype=np.float32)

    labels = np.asarray(label_table[indexes], dtype=np.int64)
    cams = np.asarray(cam_table[indexes], dtype=np.int64)

    # permute rows so camera groups are contiguous, ordered big+small so most
    # 128-row tiles span only ~2 cameras (fewer intra exp instructions)
    sizes = np.bincount(cams, minlength=C)
    order = _pair_order(sizes)
    perm = np.concatenate([np.where(cams == c)[0] for c in order])
    fp = feats[perm].astype(np.float64)
    fp = fp / np.linalg.norm(fp, axis=1, keepdims=True)
    feats_p = np.ascontiguousarray(fp, dtype=np.float32)
    labels_p = labels[perm]
    cams_p = cams[perm]
    tile_cams = tuple(
        tuple(dict.fromkeys(cams_p[128 * rt : 128 * (rt + 1)].tolist()))
        for rt in range(RT)
    )
    # camera-major padded center shards: [2, 128, 8192] per core
    cenT_shards = []
    for k in range(NCORES):
        ck = centers[k * L_LOCAL * C : (k + 1) * L_LOCAL * C]
        ck = ck.reshape(L_LOCAL, C, D).transpose(1, 0, 2)   # [C, 1000, 256]
        pad = np.zeros((C, BPAD - L_LOCAL, D), dtype=np.float32)
        ckp = np.concatenate([ck, pad], axis=1)             # [C, 1024, 256]
        cenT = ckp.reshape(P_PAD, D).T                      # [256, 8192]
        cenT_shards.append(
            np.ascontiguousarray(cenT.reshape(2, 128, P_PAD), dtype=MM_NP)
        )
    return centers, tile_cams, feats_p, labels_p, cams_p, cenT_shards


def kernel(feats, indexes, label_table, cam_table, centers):
    centers, tile_cams, feats_p, labels_p, cams_p, cenT_shards = _prepare(
        feats, indexes, label_table, cam_table, centers
    )
    nc = _build_program(tile_cams)
    runner = _get_runner(nc)
    runner.put_inputs(_make_in_maps(cenT_shards, feats_p))
    results = runner.execute()
    return _host_finish(results, feats_p, labels_p, cams_p, centers, tile_cams)


# revision 34
# speedup vs baseline: 1.1038x; 1.1038x over previous
"""Trainium2 Bass kernel for nn_CAPMemory (camera-aware proxy memory loss).

Strategy (8 NeuronCores, SPMD, no collectives):
  - Shard the 64000x256 proxy table over labels: core k owns labels
    [1000k, 1000(k+1)), all 8 cameras.  On the host the shard is laid out
    CAMERA-MAJOR with each camera block padded 1000 -> 1024 columns
    (pad centers = 0 vectors): col c*1024 + l holds proxy (label l, cam c).
    This aligns camera blocks with PSUM banks and makes the intra-camera
    softmax read contiguous.
  - Feats are normalized and transposed on the host; the device runs a pure
    pipeline: DMA -> matmul (f32r, two 128-contraction halves accumulated in
    PSUM) -> per-1024-column-unit drain -> small outputs.
  - Each (row-tile, camera-block) unit [128 x 1024] in PSUM is drained by
    exactly one engine:
      direct unit: DVE MAX8 straight from PSUM -> top-8 values (f32)
      exp unit   : ACT Exp(scale=20) from PSUM -> bf16 exp values in SBUF
                   (+ accumulated per-camera exp-sum for the intra loss)
      window unit: ACT Copy from PSUM -> bf16 sims in SBUF
    bf16 units then go through a DVE pairwise-max tree (2x perf mode) to
    8-wide window maxes [128 x 125] shipped to the host (exp-domain values
    for exp units; the host takes log/20).
  - Host merge: intra logsumexp = log(sum_k srow_k); inter top-50 hard
    negatives merged from per-block top-8s and window maxes, positives
    removed by eps value matching; positives themselves recomputed exactly
    on host in f64.
  - Certificate on direct blocks (8th value <= merged t50) triggers exact
    per-row host recomputation; window blocks are statistically covered
    (window collisions lose at most one near-cutoff negative, effect on the
    loss ~1e-4 relative, validated offline against the reference).
"""

import sys
import functools

sys.path.insert(0, "/opt/trn_rl_repo")

import numpy as np
import ml_dtypes

from concourse import bacc, mybir
from concourse.tile import TileContext

F32 = mybir.dt.float32
BF16 = mybir.dt.bfloat16

N = 512          # batch
D = 256          # feature dim
L = 8000         # labels
C = 8            # cameras
NCORES = 8
RT = 4           # row tiles of 128
L_LOCAL = 1000   # labels per core
BPAD = 1024      # padded camera-block width
P_PAD = C * BPAD  # 8192 padded columns per core
INV_T = 20.0     # 1 / temperature
K = 50           # hard negatives
LW = 0.5         # inter-cam loss weight
NW = L_LOCAL // 8  # 125 8-wide windows per camera block

# total units drained by DVE MAX8 straight from PSUM (top-8 candidates);
# the rest are drained by ACT (exp or copy) and window-max-treed on DVE.
# ACT and DVE are the only engines that can read PSUM, so this splits the
# [512 x 8192] drain across both.
DIRECT_TOTAL = 10

# matmul operand dtype: float8e4 with the DoubleRow perf mode -- both
# 128-contraction halves fold into ONE matmul at 0.5 cyc/output-col (4x the
# bf16/f32r PE rate) and the centers DMA shrinks to 2.1 MB.  Quantization
# noise on the cosine sims is ~3e-3 RMS; end-to-end loss error validated at
# ~6e-4 relative (tolerance 2e-2).  Positives are recomputed exactly on the
# host, and the eps value-matching margins below absorb the noise.
MM_DT = mybir.dt.float8e4
MM_NP = ml_dtypes.float8_e4m3


def _pair_order(sizes):
    """Order cameras big+small so most 128-row tiles span only ~2 cameras."""
    desc = np.argsort(-np.asarray(sizes), kind="stable")
    big, small = desc[: C // 2], desc[C // 2 :][::-1]
    order = []
    for b, s in zip(big, small):
        order += [int(b), int(s)]
    return order


def _unit_plan(tile_cams):
    """Static drain plan: units in b-major order, kind per unit, tree pairs.

    Returns (units, kind, pairs) where pairs maps a pair id to the list of
    its member units (1 or 2, same row-tile) and each tree unit knows its
    (pair id, slot).
    """
    units = [(rt, b) for b in range(C) for rt in range(RT)]
    kind = {}
    # distribute DIRECT_TOTAL direct units across row tiles (more directs to
    # tiles with fewer exp units)
    nexp = [len(tile_cams[rt]) for rt in range(RT)]
    d_rt = [0] * RT
    for _ in range(DIRECT_TOTAL):
        # give the next direct to the tile with most unassigned non-exp blocks
        loads = [(C - nexp[rt] - d_rt[rt], -rt) for rt in range(RT)]
        rt = -max(loads)[1]
        d_rt[rt] += 1
    for rt in range(RT):
        nonexp = [b for b in range(C) if b not in tile_cams[rt]]
        # spread the direct blocks evenly across the (b-major) drain order so
        # ACT and DVE stay concurrently busy throughout the repeat
        nd = d_rt[rt]
        directs = set(
            nonexp[int((j + 0.5) * len(nonexp) / nd)] for j in range(nd)
        ) if nd else set()
        while len(directs) < nd:  # collision fallback (never expected)
            directs.add(next(b for b in nonexp if b not in directs))
        for b in range(C):
            if b in tile_cams[rt]:
                kind[(rt, b)] = "exp"
            elif b in directs:
                kind[(rt, b)] = "direct"
            else:
                kind[(rt, b)] = "win"
    # pair tree units (exp+win) within each row tile in b order; a tree
    # unit's wm_rt slot is its position in that order, so every pair's last
    # round writes one contiguous wm_rt slice regardless of block adjacency
    pair_of = {}
    pairs = []
    wm_slot = {}
    for rt in range(RT):
        tus = [(rt, b) for b in range(C) if kind[(rt, b)] != "direct"]
        for s, u in enumerate(tus):
            wm_slot[u] = s
        for i in range(0, len(tus), 2):
            members = tus[i : i + 2]
            pid = len(pairs)
            pairs.append(members)
            for s, u in enumerate(members):
                pair_of[u] = (pid, s)
    return units, kind, pairs, pair_of, wm_slot


@functools.lru_cache(maxsize=8)
def _build_program(tile_cams, repeats=1):
    nc = bacc.Bacc(None, target_bir_lowering=False, num_swdge_queues=4)

    cenTd = nc.dram_tensor("cenT", [2, 128, P_PAD], MM_DT, kind="ExternalInput")
    fTd = nc.dram_tensor("fT", [128, RT, 2, 128], MM_DT, kind="ExternalInput")
    candd = nc.dram_tensor("cand", [RT, 128, C * 8], F32, kind="ExternalOutput")
    srowd = nc.dram_tensor("srow", [RT, 128, C], F32, kind="ExternalOutput")
    wmaxd = nc.dram_tensor("wmax", [RT, 128, C, NW], BF16, kind="ExternalOutput")

    with TileContext(nc) as tc:
        with (
            tc.tile_pool(name="cen", bufs=2) as cenp,
            tc.tile_pool(name="ftp", bufs=2) as ftp,
            tc.tile_pool(name="scrp", bufs=5) as scrp,
            tc.tile_pool(name="treep", bufs=3) as treep,
            tc.tile_pool(name="outp", bufs=2) as outp,
            tc.tile_pool(name="psum", bufs=4, space="PSUM") as psump,
        ):
            for _rep in range(repeats):
                _kernel_body(nc, tc, cenp, ftp, scrp, treep, outp, psump,
                             cenTd, fTd, candd, srowd, wmaxd, tile_cams)

    nc.compile()
    return nc


def _kernel_body(nc, tc, cenp, ftp, scrp, treep, outp, psump,
                 cenTd, fTd, candd, srowd, wmaxd, tile_cams):
    ActF = mybir.ActivationFunctionType

    units, kind, pairs, pair_of, wm_slot = _unit_plan(tile_cams)

    # ---- input DMA: fT first (matmuls need it immediately), then centers in
    # (h, block) granularity so early units unblock fast; alternate issuing
    # engines to spread descriptor generation across queues
    # inputs on SP only (so the next repeat's loads never queue behind this
    # repeat's output descriptors); outputs on gpsimd only
    fT_sb = ftp.tile([128, RT, 2, 128], MM_DT, name="fT_sb")
    nc.sync.dma_start(out=fT_sb[:, :, :, :], in_=fTd[:, :, :, :])
    cen_sb = cenp.tile([128, 2, P_PAD], MM_DT, name="cen_sb")
    for b in range(C):
        sl = slice(b * BPAD, (b + 1) * BPAD)
        for h in range(2):
            nc.sync.dma_start(out=cen_sb[:, h, sl], in_=cenTd[h, :, sl])

    cand_sb = [outp.tile([128, C * 8], F32, name=f"cand{rt}", bufs=2)
               for rt in range(RT)]
    s_t = [outp.tile([128, C], F32, name=f"st{rt}", bufs=2)
           for rt in range(RT)]
    wm_rt = [outp.tile([128, C, NW], BF16, name=f"wm{rt}", bufs=2)
             for rt in range(RT)]

    # pair state: scr tiles allocated lazily, members drain at different times
    pair_scr = [None] * len(pairs)
    pair_filled = [0] * len(pairs)
    # how many direct/exp/tree units remain per rt (to time the output DMAs)
    left_direct = [sum(1 for b in range(C) if kind[(rt, b)] == "direct")
                   for rt in range(RT)]
    left_exp = [len(tile_cams[rt]) for rt in range(RT)]
    left_tree = [sum(1 for b in range(C) if kind[(rt, b)] != "direct")
                 for rt in range(RT)]

    for ui, (rt, b) in enumerate(units):
        ps = psump.tile([128, BPAD], F32, name="ps")
        c0 = b * BPAD
        for j in range(2):
            nc.tensor.matmul(
                ps[:, j * 512 : (j + 1) * 512],
                fT_sb[:, rt, :, :],
                cen_sb[:, :, c0 + j * 512 : c0 + (j + 1) * 512],
                start=True, stop=True,
                perf_mode=mybir.MatmulPerfMode.DoubleRow,
            )
        k = kind[(rt, b)]
        if k == "direct":
            nc.vector.max(cand_sb[rt][:, b * 8 : b * 8 + 8], ps[:, 0:L_LOCAL])
            left_direct[rt] -= 1
            if left_direct[rt] == 0:
                nc.gpsimd.dma_start(out=candd[rt], in_=cand_sb[rt][:, :])
            continue
        pid, slot = pair_of[(rt, b)]
        npair = len(pairs[pid])
        if pair_scr[pid] is None:
            pair_scr[pid] = scrp.tile([128, npair, L_LOCAL], BF16, name="scr")
        scr = pair_scr[pid]
        if k == "exp":
            idx = tile_cams[rt].index(b)
            nc.scalar.activation(
                scr[:, slot, :], ps[:, 0:L_LOCAL], ActF.Exp,
                scale=INV_T, accum_out=s_t[rt][:, idx : idx + 1],
            )
            left_exp[rt] -= 1
            if left_exp[rt] == 0:
                nc.gpsimd.dma_start(out=srowd[rt], in_=s_t[rt][:, :])
        else:
            nc.scalar.copy(scr[:, slot, :], ps[:, 0:L_LOCAL])
        pair_filled[pid] += 1
        if pair_filled[pid] == npair:
            # bf16 pairwise-max tree: [np,125,8] -> per-block window maxes;
            # when the pair's blocks are adjacent the last round writes one
            # contiguous wm_rt slice, otherwise one instruction per member
            v = scr.rearrange("p np (nw w) -> p np nw w", w=8)
            t1 = treep.tile([128, npair, NW, 4], BF16, name="t1")
            t2 = treep.tile([128, npair, NW, 2], BF16, name="t2")
            nc.vector.tensor_max(t1[:, :, :, :], v[:, :, :, 0:4],
                                 v[:, :, :, 4:8])
            nc.vector.tensor_max(t2[:, :, :, :], t1[:, :, :, 0:2],
                                 t1[:, :, :, 2:4])
            dst = wm_rt[rt].rearrange("p c (nw one) -> p c nw one", one=1)
            s0 = wm_slot[pairs[pid][0]]
            nc.vector.tensor_max(dst[:, s0 : s0 + npair, :, :],
                                 t2[:, :, :, 0:1], t2[:, :, :, 1:2])
            left_tree[rt] -= npair
            if left_tree[rt] == 0:
                nc.gpsimd.dma_start(out=wmaxd[rt], in_=wm_rt[rt][:, :, :])


class _Runner:
    """Sharded 8-core executor for a built Bass program.

    Builds the jax.jit(shard_map(bass_exec)) executable once (the walrus/NEFF
    compile happens inside the first call) and reuses it for every subsequent
    execution, keeping large inputs device-resident.
    """

    def __init__(self, nc, n_cores=NCORES):
        import jax
        from jax.sharding import Mesh, PartitionSpec, NamedSharding
        from jax.experimental.shard_map import shard_map
        from concourse import bass2jax

        self.jax = jax
        self.nc = nc
        self.n_cores = n_cores
        bass2jax.install_neuronx_cc_hook()
        partition_name = (
            nc.partition_id_tensor.name if nc.partition_id_tensor else None
        )
        in_names, out_names, out_avals = [], [], []
        for alloc in nc.m.functions[0].allocations:
            if not isinstance(alloc, mybir.MemoryLocationSet):
                continue
            name = alloc.memorylocations[0].name
            if alloc.kind == "ExternalInput":
                if name != partition_name:
                    in_names.append(name)
            elif alloc.kind == "ExternalOutput":
                out_names.append(name)
                out_avals.append(
                    jax.core.ShapedArray(
                        tuple(alloc.tensor_shape), mybir.dt.np(alloc.dtype)
                    )
                )
        self.in_names, self.out_names, self.out_avals = in_names, out_names, out_avals
        n_params, n_outs = len(in_names), len(out_avals)
        all_in_names = list(in_names) + list(out_names)
        if partition_name is not None:
            all_in_names.append(partition_name)

        def _body(*args):
            operands = list(args)
            if partition_name is not None:
                operands.append(bass2jax.partition_id_tensor())
            return tuple(
                bass2jax._bass_exec_p.bind(
                    *operands,
                    out_avals=tuple(out_avals),
                    in_names=tuple(all_in_names),
                    out_names=tuple(out_names),
                    lowering_input_output_aliases=(),
                    sim_require_finite=True,
                    sim_require_nnan=True,
                    nc=nc,
                )
            )

        devices = jax.devices()[:n_cores]
        self.mesh = Mesh(np.asarray(devices), ("core",))
        self.sh = NamedSharding(self.mesh, PartitionSpec("core"))
        self.fn = jax.jit(
            shard_map(
                _body,
                mesh=self.mesh,
                in_specs=(PartitionSpec("core"),) * (n_params + n_outs),
                out_specs=(PartitionSpec("core"),) * n_outs,
                check_rep=False,
            ),
            donate_argnums=tuple(range(n_params, n_params + n_outs)),
            keep_unused=True,
        )
        self._zero_shapes = [
            ((n_cores * a.shape[0], *a.shape[1:]), a.dtype) for a in out_avals
        ]

    def put_inputs(self, in_maps):
        self.dev_in = [
            self.jax.device_put(
                np.concatenate([np.asarray(m[name]) for m in in_maps], axis=0),
                self.sh,
            )
            for name in self.in_names
        ]

    def _zeros(self):
        return [
            self.jax.device_put(np.zeros(s, d), self.sh)
            for s, d in self._zero_shapes
        ]

    def execute(self):
        outs = self.fn(*self.dev_in, *self._zeros())
        self.jax.block_until_ready(outs)
        return self.unpack(outs)

    def unpack(self, outs):
        return [
            {
                name: np.asarray(outs[i]).reshape(
                    self.n_cores, *self.out_avals[i].shape
                )[c]
                for i, name in enumerate(self.out_names)
            }
            for c in range(self.n_cores)
        ]


_RUNNERS = {}
_LAST_FALLBACKS = 0
_FORCE_FALLBACK = False  # test hook: exercise the exact host fallback path


def _get_runner(nc):
    r = _RUNNERS.get(id(nc))
    if r is None:
        r = _Runner(nc)
        _RUNNERS[id(nc)] = r
    return r


def _make_in_maps(cenT_shards, feats_p):
    # feats_p is the permuted, L2-normalized batch; device wants the
    # transposed layout [q, rt, h, r] with q the contraction partition
    fT = np.ascontiguousarray(
        feats_p.reshape(RT, 128, 2, 128).transpose(3, 0, 2, 1), dtype=MM_NP
    )
    return [
        {"cenT": np.ascontiguousarray(cenT_shards[k], dtype=MM_NP), "fT": fT}
        for k in range(NCORES)
    ]


def _host_finish(results, feats_p, labels_p, cams_p, centers, tile_cams):
    units, kind, pairs, pair_of, wm_slot = _unit_plan(tile_cams)
    rows = np.arange(N)
    rt_of = rows // 128
    p_of = rows % 128

    # ---- intra: sum over cores of per-camera exp sums ----
    slot = np.zeros(N, dtype=np.int64)
    for rt in range(RT):
        for idx, cam in enumerate(tile_cams[rt]):
            sel = slice(128 * rt, 128 * (rt + 1))
            slot[sel] = np.where(cams_p[sel] == cam, idx, slot[sel])
    s_k = np.stack(
        [
            results[k]["srow"].reshape(RT, 128, C)[rt_of, p_of, slot]
            for k in range(NCORES)
        ]
    ).astype(np.float64)  # [8, 512]: sum_l exp(20 * cos sims) per core

    fn = feats_p.astype(np.float64)
    fn = fn / np.linalg.norm(fn, axis=1, keepdims=True)
    cen = centers.astype(np.float64)
    gidx = labels_p[:, None] * C + np.arange(C)[None, :]        # [512, 8]
    pos = np.einsum("rcd,rd->rc", cen[gidx], fn)                # [512, 8] f64

    lse_intra = np.log(s_k.sum(axis=0))
    v = pos[rows, cams_p]
    loss_intra_i = lse_intra - INV_T * v

    # ---- inter: merge candidates ----
    # direct blocks contribute their top-8 values; tree blocks their 125
    # window maxes (exp blocks in exp domain: s = log(w)/20).
    cand = np.stack([results[k]["cand"] for k in range(NCORES)])  # [8,RT,128,64]
    wmraw = np.stack(
        [results[k]["wmax"].astype(np.float32) for k in range(NCORES)]
    )  # [8,RT,128,C,NW], slot-indexed per row tile

    # remap slots -> blocks, converting exp-domain window maxes back to sims
    # domain; direct blocks have no window data (-inf)
    wmax = np.full_like(wmraw, -np.inf)
    for rt in range(RT):
        for b in range(C):
            kd = kind[(rt, b)]
            if kd == "direct":
                continue
            w = wmraw[:, rt, :, wm_slot[(rt, b)], :]
            if kd == "exp":
                w = np.log(np.maximum(w, 1e-30)) / INV_T
            wmax[:, rt, :, b, :] = w

    wspan = C * NW
    cspan = C * 8
    span = wspan + cspan
    CR = np.empty((N, NCORES * span), dtype=np.float64)
    # window part: CR[i, k*span + b*NW + w]
    CR[:, : NCORES * wspan].reshape(N, NCORES, wspan)[:] = (
        wmax[:, rt_of, p_of, :, :].transpose(1, 0, 2, 3).reshape(N, NCORES, wspan)
    )
    # direct part: CR[i, NCORES*wspan + k*cspan + b*8 + j], -inf for non-direct
    cpart = cand[:, rt_of, p_of, :].transpose(1, 0, 2)          # [512, 8, 64]
    dmask = np.zeros((N, cspan), dtype=bool)
    for rt in range(RT):
        rsel = rt_of == rt
        for b in range(C):
            if kind[(rt, b)] == "direct":
                dmask[rsel, b * 8 : b * 8 + 8] = True
    CR[:, NCORES * wspan :] = np.where(
        dmask[:, None, :], cpart, -np.inf
    ).reshape(N, NCORES * cspan)

    # ---- remove positives by eps value matching ----
    owner = labels_p // L_LOCAL
    lloc = labels_p % L_LOCAL
    win = lloc // 8
    EPS = 1.5e-2
    for i in rows:
        rt = rt_of[i]
        k0 = owner[i]
        for c in range(C):
            if kind[(rt, c)] == "direct":
                idxs = np.arange(NCORES * wspan + k0 * cspan + c * 8,
                                 NCORES * wspan + k0 * cspan + c * 8 + 8)
                vals = CR[i, idxs]
                j = int(np.argmin(np.abs(vals - pos[i, c])))
                if abs(vals[j] - pos[i, c]) < EPS:
                    CR[i, idxs[j]] = -np.inf
            else:
                jj = k0 * wspan + c * NW + win[i]
                if abs(CR[i, jj] - pos[i, c]) < EPS:
                    CR[i, jj] = -np.inf

    part = np.partition(CR, CR.shape[1] - K, axis=1)[:, -K:]
    t50 = part.min(axis=1)

    # ---- certificate on direct blocks: 8th value must be <= t50 ----
    if _FORCE_FALLBACK:
        bad = rows
    else:
        worst = np.full(N, -np.inf)
        for rt in range(RT):
            rsel = rt_of == rt
            for b in range(C):
                if kind[(rt, b)] != "direct":
                    continue
                worst[rsel] = np.maximum(
                    worst[rsel],
                    cand[:, rt, :, b * 8 + 7].max(axis=0)[p_of[rsel]],
                )
        bad = np.where(worst > t50)[0]
    global _LAST_FALLBACKS
    _LAST_FALLBACKS = len(bad)
    for i in bad:
        sims_row = cen @ fn[i]                                  # [64000] exact
        sims_row[C * labels_p[i] : C * labels_p[i] + C] = -np.inf
        part[i] = np.sort(sims_row)[-K:]

    z = np.concatenate([pos, part], axis=1) * INV_T             # [512, 58]
    mz = z.max(axis=1)
    lse_inter = np.log(np.exp(z - mz[:, None]).sum(axis=1)) + mz
    loss_inter_i = lse_inter - INV_T * pos.mean(axis=1)

    # ---- per-camera means, summed ----
    cnt = np.bincount(cams_p, minlength=C).astype(np.float64)
    s_intra = np.bincount(cams_p, weights=loss_intra_i, minlength=C)
    s_inter = np.bincount(cams_p, weights=loss_inter_i, minlength=C)
    safe = np.maximum(cnt, 1.0)
    li = np.sum(np.where(cnt > 0, s_intra / safe, 0.0))
    le = LW * np.sum(np.where(cnt > 0, s_inter / safe, 0.0))
    return np.array([li, le], dtype=np.float32)


def _prepare(feats, indexes, label_table, cam_table, centers):
    feats = np.asarray(feats, dtype=np.float32)
    indexes = np.asarray(indexes)
    label_table = np.asarray(label_table)
    cam_table = np.asarray(cam_table)
    centers = np.asarray(centers, dtype=np.float32)

    labels = np.asarray(label_table[indexes], dtype=np.int64)
    cams = np.asarray(cam_table[indexes], dtype=np.int64)

    # permute rows so camera groups are contiguous, ordered big+small so most
    # 128-row tiles span only ~2 cameras (fewer intra exp instructions)
    sizes = np.bincount(cams, minlength=C)
    order = _pair_order(sizes)
    perm = np.concatenate([np.where(cams == c)[0] for c in order])
    fp = feats[perm].astype(np.float64)
    fp = fp / np.linalg.norm(fp, axis=1, keepdims=True)
    feats_p = np.ascontiguousarray(fp, dtype=np.float32)
    labels_p = labels[perm]
    cams_p = cams[perm]
    tile_cams = tuple(
        tuple(dict.fromkeys(cams_p[128 * rt : 128 * (rt + 1)].tolist()))
        for rt in range(RT)
    )
    # camera-major padded center shards: [2, 128, 8192] per core
    cenT_shards = []
    for k in range(NCORES):
        ck = centers[k * L_LOCAL * C : (k + 1) * L_LOCAL * C]
        ck = ck.reshape(L_LOCAL, C, D).transpose(1, 0, 2)   # [C, 1000, 256]
        pad = np.zeros((C, BPAD - L_LOCAL, D), dtype=np.float32)
        ckp = np.concatenate([ck, pad], axis=1)             # [C, 1024, 256]
        cenT = ckp.reshape(P_PAD, D).T                      # [256, 8192]
        cenT_shards.append(
            np.ascontiguousarray(cenT.reshape(2, 128, P_PAD), dtype=MM_NP)
        )
    return centers, tile_cams, feats_p, labels_p, cams_p, cenT_shards


def kernel(feats, indexes, label_table, cam_table, centers):
    centers, tile_cams, feats_p, labels_p, cams_p, cenT_shards = _prepare(
        feats, indexes, label_table, cam_table, centers
    )
    nc = _build_program(tile_cams)
    runner = _get_runner(nc)
    runner.put_inputs(_make_in_maps(cenT_shards, feats_p))
    results = runner.execute()
    return _host_finish(results, feats_p, labels_p, cams_p, centers, tile_cams)


# revision 37
# speedup vs baseline: 1.1222x; 1.0167x over previous
"""Trainium2 Bass kernel for nn_CAPMemory (camera-aware proxy memory loss).

Strategy (8 NeuronCores, SPMD, no collectives):
  - Shard the 64000x256 proxy table over labels: core k owns labels
    [1000k, 1000(k+1)), all 8 cameras.  On the host the shard is laid out
    CAMERA-MAJOR with each camera block padded 1000 -> 1024 columns
    (pad centers = 0 vectors): col c*1024 + l holds proxy (label l, cam c).
    This aligns camera blocks with PSUM banks and makes the intra-camera
    softmax read contiguous.
  - Feats are normalized and transposed on the host; the device runs a pure
    pipeline: DMA -> fp8e4 DoubleRow matmul (both 128-contraction halves in
    one instruction) -> per-1024-column-unit PSUM drain -> small outputs.
  - Each (row-tile, camera-block) unit [128 x 1024] in PSUM is drained by
    exactly one engine:
      direct unit: DVE MAX8 straight from PSUM -> top-8 values (f32)
      exp unit   : ACT Exp(scale=20) from PSUM -> bf16 exp values in SBUF
                   (+ accumulated per-camera exp-sum for the intra loss)
      window unit: ACT Copy from PSUM -> bf16 sims in SBUF
    bf16 units then go through a DVE pairwise-max tree (2x perf mode) to
    8-wide window maxes [128 x 125] shipped to the host (exp-domain values
    for exp units; the host takes log/20).
  - Host merge: intra logsumexp = log(sum_k srow_k); inter top-50 hard
    negatives merged from per-block top-8s and window maxes, positives
    removed by eps value matching; positives themselves recomputed exactly
    on host in f64.
  - Certificate on direct blocks (8th value <= merged t50) triggers exact
    per-row host recomputation; window blocks are statistically covered
    (window collisions lose at most one near-cutoff negative, effect on the
    loss ~1e-4 relative, validated offline against the reference).
"""

import sys
import functools

sys.path.insert(0, "/opt/trn_rl_repo")

import numpy as np
import ml_dtypes

from concourse import bacc, mybir
from concourse.tile import TileContext

F32 = mybir.dt.float32
BF16 = mybir.dt.bfloat16

N = 512          # batch
D = 256          # feature dim
L = 8000         # labels
C = 8            # cameras
NCORES = 8
RT = 4           # row tiles of 128
L_LOCAL = 1000   # labels per core
BPAD = 1024      # padded camera-block width
P_PAD = C * BPAD  # 8192 padded columns per core
INV_T = 20.0     # 1 / temperature
K = 50           # hard negatives
LW = 0.5         # inter-cam loss weight
NW = L_LOCAL // 8  # 125 8-wide windows per camera block

# total units drained by DVE MAX8 straight from PSUM (top-8 candidates);
# the rest are drained by ACT (exp or copy) and window-max-treed on DVE.
# ACT and DVE are the only engines that can read PSUM, so this splits the
# [512 x 8192] drain across both.
DIRECT_TOTAL = 10

# matmul operand dtype: float8e4 with the DoubleRow perf mode -- both
# 128-contraction halves fold into ONE matmul at 0.5 cyc/output-col (4x the
# bf16/f32r PE rate) and the centers DMA shrinks to 2.1 MB.  Quantization
# noise on the cosine sims is ~3e-3 RMS; end-to-end loss error validated at
# ~6e-4 relative (tolerance 2e-2).  Positives are recomputed exactly on the
# host, and the eps value-matching margins below absorb the noise.
MM_DT = mybir.dt.float8e4
MM_NP = ml_dtypes.float8_e4m3


def _pair_order(sizes):
    """Order cameras big+small so most 128-row tiles span only ~2 cameras."""
    desc = np.argsort(-np.asarray(sizes), kind="stable")
    big, small = desc[: C // 2], desc[C // 2 :][::-1]
    order = []
    for b, s in zip(big, small):
        order += [int(b), int(s)]
    return order


def _unit_plan(tile_cams):
    """Static drain plan: units in b-major order, kind per unit, tree pairs.

    Returns (units, kind, pairs, pair_of, wm_slot): pairs maps a pair id to
    its member units (1 or 2, same row tile); pair_of gives each tree unit
    its (pair id, member slot); wm_slot gives each tree unit its slot in the
    row tile's wm output (pair-order, so tree last rounds are contiguous).
    """
    units = [(rt, b) for b in range(C) for rt in range(RT)]
    kind = {}
    # distribute DIRECT_TOTAL direct units across row tiles (more directs to
    # tiles with fewer exp units)
    nexp = [len(tile_cams[rt]) for rt in range(RT)]
    d_rt = [0] * RT
    for _ in range(DIRECT_TOTAL):
        # give the next direct to the tile with most unassigned non-exp blocks
        loads = [(C - nexp[rt] - d_rt[rt], -rt) for rt in range(RT)]
        rt = -max(loads)[1]
        d_rt[rt] += 1
    # place the direct units so every b-column of the (b-major) drain order
    # gets ~one DVE-drained unit (rotating the row tile): ACT and DVE then
    # stay concurrently busy instead of alternating in bursts
    quota = list(d_rt)
    directs = set()
    for b in range(C):
        cands = [rt for rt in range(RT)
                 if b not in tile_cams[rt] and quota[rt] > 0]
        if not cands:
            continue
        rt = sorted(cands, key=lambda r: (-quota[r], (r - b) % RT))[0]
        directs.add((rt, b))
        quota[rt] -= 1
    for rt in range(RT):
        while quota[rt] > 0:
            ncol = {b: sum(1 for (r2, b2) in directs if b2 == b)
                    for b in range(C)}
            free = [b for b in range(C)
                    if b not in tile_cams[rt] and (rt, b) not in directs]
            b = sorted(free, key=lambda b2: (ncol[b2], b2))[0]
            directs.add((rt, b))
            quota[rt] -= 1
    for rt in range(RT):
        for b in range(C):
            if b in tile_cams[rt]:
                kind[(rt, b)] = "exp"
            elif (rt, b) in directs:
                kind[(rt, b)] = "direct"
            else:
                kind[(rt, b)] = "win"
    # pair tree units (exp+win) within each row tile in b order; a tree
    # unit's wm_rt slot is its position in that order, so every pair's last
    # round writes one contiguous wm_rt slice regardless of block adjacency
    pair_of = {}
    pairs = []
    wm_slot = {}
    for rt in range(RT):
        tus = [(rt, b) for b in range(C) if kind[(rt, b)] != "direct"]
        for s, u in enumerate(tus):
            wm_slot[u] = s
        for i in range(0, len(tus), 2):
            members = tus[i : i + 2]
            pid = len(pairs)
            pairs.append(members)
            for s, u in enumerate(members):
                pair_of[u] = (pid, s)
    return units, kind, pairs, pair_of, wm_slot


@functools.lru_cache(maxsize=8)
def _build_program(tile_cams, repeats=1):
    nc = bacc.Bacc(None, target_bir_lowering=False, num_swdge_queues=4)

    cenTd = nc.dram_tensor("cenT", [2, 128, P_PAD], MM_DT, kind="ExternalInput")
    fTd = nc.dram_tensor("fT", [128, RT, 2, 128], MM_DT, kind="ExternalInput")
    candd = nc.dram_tensor("cand", [RT, 128, C * 8], F32, kind="ExternalOutput")
    srowd = nc.dram_tensor("srow", [RT, 128, C], F32, kind="ExternalOutput")
    wmaxd = nc.dram_tensor("wmax", [RT, 128, C, NW], BF16, kind="ExternalOutput")

    with TileContext(nc) as tc:
        with (
            tc.tile_pool(name="cen", bufs=2) as cenp,
            tc.tile_pool(name="ftp", bufs=2) as ftp,
            tc.tile_pool(name="scrp", bufs=5) as scrp,
            tc.tile_pool(name="treep", bufs=3) as treep,
            tc.tile_pool(name="outp", bufs=2) as outp,
            tc.tile_pool(name="psum", bufs=4, space="PSUM") as psump,
        ):
            for _rep in range(repeats):
                _kernel_body(nc, tc, cenp, ftp, scrp, treep, outp, psump,
                             cenTd, fTd, candd, srowd, wmaxd, tile_cams)

    nc.compile()
    return nc


def _kernel_body(nc, tc, cenp, ftp, scrp, treep, outp, psump,
                 cenTd, fTd, candd, srowd, wmaxd, tile_cams):
    ActF = mybir.ActivationFunctionType

    units, kind, pairs, pair_of, wm_slot = _unit_plan(tile_cams)

    # ---- input DMA: fT first (matmuls need it immediately), then centers in
    # (h, block) granularity so early units unblock fast; alternate issuing
    # engines to spread descriptor generation across queues
    # inputs on SP only (so the next repeat's loads never queue behind this
    # repeat's output descriptors); outputs on gpsimd only
    fT_sb = ftp.tile([128, RT, 2, 128], MM_DT, name="fT_sb")
    nc.sync.dma_start(out=fT_sb[:, :, :, :], in_=fTd[:, :, :, :])
    cen_sb = cenp.tile([128, 2, P_PAD], MM_DT, name="cen_sb")
    for b in range(C):
        sl = slice(b * BPAD, (b + 1) * BPAD)
        for h in range(2):
            nc.sync.dma_start(out=cen_sb[:, h, sl], in_=cenTd[h, :, sl])

    cand_sb = [outp.tile([128, C * 8], F32, name=f"cand{rt}", bufs=2)
               for rt in range(RT)]
    s_t = [outp.tile([128, C], F32, name=f"st{rt}", bufs=2)
           for rt in range(RT)]
    wm_rt = [outp.tile([128, C, NW], BF16, name=f"wm{rt}", bufs=2)
             for rt in range(RT)]

    # pair state: scr tiles allocated lazily, members drain at different times
    pair_scr = [None] * len(pairs)
    pair_filled = [0] * len(pairs)
    # how many direct/exp/tree units remain per rt (to time the output DMAs)
    left_direct = [sum(1 for b in range(C) if kind[(rt, b)] == "direct")
                   for rt in range(RT)]
    left_exp = [len(tile_cams[rt]) for rt in range(RT)]
    left_tree = [sum(1 for b in range(C) if kind[(rt, b)] != "direct")
                 for rt in range(RT)]

    for ui, (rt, b) in enumerate(units):
        ps = psump.tile([128, BPAD], F32, name="ps")
        c0 = b * BPAD
        for j in range(2):
            nc.tensor.matmul(
                ps[:, j * 512 : (j + 1) * 512],
                fT_sb[:, rt, :, :],
                cen_sb[:, :, c0 + j * 512 : c0 + (j + 1) * 512],
                start=True, stop=True,
                perf_mode=mybir.MatmulPerfMode.DoubleRow,
            )
        k = kind[(rt, b)]
        if k == "direct":
            nc.vector.max(cand_sb[rt][:, b * 8 : b * 8 + 8], ps[:, 0:L_LOCAL])
            left_direct[rt] -= 1
            if left_direct[rt] == 0:
                nc.gpsimd.dma_start(out=candd[rt], in_=cand_sb[rt][:, :])
            continue
        pid, slot = pair_of[(rt, b)]
        npair = len(pairs[pid])
        if pair_scr[pid] is None:
            pair_scr[pid] = scrp.tile([128, npair, L_LOCAL], BF16, name="scr")
        scr = pair_scr[pid]
        if k == "exp":
            idx = tile_cams[rt].index(b)
            nc.scalar.activation(
                scr[:, slot, :], ps[:, 0:L_LOCAL], ActF.Exp,
                scale=INV_T, accum_out=s_t[rt][:, idx : idx + 1],
            )
            left_exp[rt] -= 1
            if left_exp[rt] == 0:
                nc.gpsimd.dma_start(out=srowd[rt], in_=s_t[rt][:, :])
        else:
            nc.scalar.copy(scr[:, slot, :], ps[:, 0:L_LOCAL])
        pair_filled[pid] += 1
        if pair_filled[pid] == npair:
            # bf16 pairwise-max tree: [np,125,8] -> per-block window maxes;
            # when the pair's blocks are adjacent the last round writes one
            # contiguous wm_rt slice, otherwise one instruction per member
            v = scr.rearrange("p np (nw w) -> p np nw w", w=8)
            t1 = treep.tile([128, npair, NW, 4], BF16, name="t1")
            t2 = treep.tile([128, npair, NW, 2], BF16, name="t2")
            nc.vector.tensor_max(t1[:, :, :, :], v[:, :, :, 0:4],
                                 v[:, :, :, 4:8])
            nc.vector.tensor_max(t2[:, :, :, :], t1[:, :, :, 0:2],
                                 t1[:, :, :, 2:4])
            dst = wm_rt[rt].rearrange("p c (nw one) -> p c nw one", one=1)
            s0 = wm_slot[pairs[pid][0]]
            nc.vector.tensor_max(dst[:, s0 : s0 + npair, :, :],
                                 t2[:, :, :, 0:1], t2[:, :, :, 1:2])
            left_tree[rt] -= npair
            if left_tree[rt] == 0:
                nc.gpsimd.dma_start(out=wmaxd[rt], in_=wm_rt[rt][:, :, :])


class _Runner:
    """Sharded 8-core executor for a built Bass program.

    Builds the jax.jit(shard_map(bass_exec)) executable once (the walrus/NEFF
    compile happens inside the first call) and reuses it for every subsequent
    execution, keeping large inputs device-resident.
    """

    def __init__(self, nc, n_cores=NCORES):
        import jax
        from jax.sharding import Mesh, PartitionSpec, NamedSharding
        from jax.experimental.shard_map import shard_map
        from concourse import bass2jax

        self.jax = jax
        self.nc = nc
        self.n_cores = n_cores
        bass2jax.install_neuronx_cc_hook()
        partition_name = (
            nc.partition_id_tensor.name if nc.partition_id_tensor else None
        )
        in_names, out_names, out_avals = [], [], []
        for alloc in nc.m.functions[0].allocations:
            if not isinstance(alloc, mybir.MemoryLocationSet):
                continue
            name = alloc.memorylocations[0].name
            if alloc.kind == "ExternalInput":
                if name != partition_name:
                    in_names.append(name)
            elif alloc.kind == "ExternalOutput":
                out_names.append(name)
                out_avals.append(
                    jax.core.ShapedArray(
                        tuple(alloc.tensor_shape), mybir.dt.np(alloc.dtype)
                    )
                )
        self.in_names, self.out_names, self.out_avals = in_names, out_names, out_avals
        n_params, n_outs = len(in_names), len(out_avals)
        all_in_names = list(in_names) + list(out_names)
        if partition_name is not None:
            all_in_names.append(partition_name)

        def _body(*args):
            operands = list(args)
            if partition_name is not None:
                operands.append(bass2jax.partition_id_tensor())
            return tuple(
                bass2jax._bass_exec_p.bind(
                    *operands,
                    out_avals=tuple(out_avals),
                    in_names=tuple(all_in_names),
                    out_names=tuple(out_names),
                    lowering_input_output_aliases=(),
                    sim_require_finite=True,
                    sim_require_nnan=True,
                    nc=nc,
                )
            )

        devices = jax.devices()[:n_cores]
        self.mesh = Mesh(np.asarray(devices), ("core",))
        self.sh = NamedSharding(self.mesh, PartitionSpec("core"))
        self.fn = jax.jit(
            shard_map(
                _body,
                mesh=self.mesh,
                in_specs=(PartitionSpec("core"),) * (n_params + n_outs),
                out_specs=(PartitionSpec("core"),) * n_outs,
                check_rep=False,
            ),
            donate_argnums=tuple(range(n_params, n_params + n_outs)),
            keep_unused=True,
        )
        self._zero_shapes = [
            ((n_cores * a.shape[0], *a.shape[1:]), a.dtype) for a in out_avals
        ]

    def put_inputs(self, in_maps):
        self.dev_in = [
            self.jax.device_put(
                np.concatenate([np.asarray(m[name]) for m in in_maps], axis=0),
                self.sh,
            )
            for name in self.in_names
        ]

    def _zeros(self):
        return [
            self.jax.device_put(np.zeros(s, d), self.sh)
            for s, d in self._zero_shapes
        ]

    def execute(self):
        outs = self.fn(*self.dev_in, *self._zeros())
        self.jax.block_until_ready(outs)
        return self.unpack(outs)

    def unpack(self, outs):
        return [
            {
                name: np.asarray(outs[i]).reshape(
                    self.n_cores, *self.out_avals[i].shape
                )[c]
                for i, name in enumerate(self.out_names)
            }
            for c in range(self.n_cores)
        ]


_RUNNERS = {}
_LAST_FALLBACKS = 0
_FORCE_FALLBACK = False  # test hook: exercise the exact host fallback path


def _get_runner(nc):
    r = _RUNNERS.get(id(nc))
    if r is None:
        r = _Runner(nc)
        _RUNNERS[id(nc)] = r
    return r


def _make_in_maps(cenT_shards, feats_p):
    # feats_p is the permuted, L2-normalized batch; device wants the
    # transposed layout [q, rt, h, r] with q the contraction partition
    fT = np.ascontiguousarray(
        feats_p.reshape(RT, 128, 2, 128).transpose(3, 0, 2, 1), dtype=MM_NP
    )
    return [
        {"cenT": np.ascontiguousarray(cenT_shards[k], dtype=MM_NP), "fT": fT}
        for k in range(NCORES)
    ]


def _host_finish(results, feats_p, labels_p, cams_p, centers, tile_cams):
    units, kind, pairs, pair_of, wm_slot = _unit_plan(tile_cams)
    rows = np.arange(N)
    rt_of = rows // 128
    p_of = rows % 128

    # ---- intra: sum over cores of per-camera exp sums ----
    slot = np.zeros(N, dtype=np.int64)
    for rt in range(RT):
        for idx, cam in enumerate(tile_cams[rt]):
            sel = slice(128 * rt, 128 * (rt + 1))
            slot[sel] = np.where(cams_p[sel] == cam, idx, slot[sel])
    s_k = np.stack(
        [
            results[k]["srow"].reshape(RT, 128, C)[rt_of, p_of, slot]
            for k in range(NCORES)
        ]
    ).astype(np.float64)  # [8, 512]: sum_l exp(20 * cos sims) per core

    fn = feats_p.astype(np.float64)
    fn = fn / np.linalg.norm(fn, axis=1, keepdims=True)
    cen = centers.astype(np.float64)
    gidx = labels_p[:, None] * C + np.arange(C)[None, :]        # [512, 8]
    pos = np.einsum("rcd,rd->rc", cen[gidx], fn)                # [512, 8] f64

    lse_intra = np.log(s_k.sum(axis=0))
    v = pos[rows, cams_p]
    loss_intra_i = lse_intra - INV_T * v

    # ---- inter: merge candidates ----
    # direct blocks contribute their top-8 values; tree blocks their 125
    # window maxes (exp blocks in exp domain: s = log(w)/20).
    cand = np.stack([results[k]["cand"] for k in range(NCORES)])  # [8,RT,128,64]
    wmraw = np.stack(
        [results[k]["wmax"].astype(np.float32) for k in range(NCORES)]
    )  # [8,RT,128,C,NW], slot-indexed per row tile

    # remap slots -> blocks, converting exp-domain window maxes back to sims
    # domain; direct blocks have no window data (-inf)
    wmax = np.full_like(wmraw, -np.inf)
    for rt in range(RT):
        for b in range(C):
            kd = kind[(rt, b)]
            if kd == "direct":
                continue
            w = wmraw[:, rt, :, wm_slot[(rt, b)], :]
            if kd == "exp":
                w = np.log(np.maximum(w, 1e-30)) / INV_T
            wmax[:, rt, :, b, :] = w

    wspan = C * NW
    cspan = C * 8
    span = wspan + cspan
    CR = np.empty((N, NCORES * span), dtype=np.float64)
    # window part: CR[i, k*span + b*NW + w]
    CR[:, : NCORES * wspan].reshape(N, NCORES, wspan)[:] = (
        wmax[:, rt_of, p_of, :, :].transpose(1, 0, 2, 3).reshape(N, NCORES, wspan)
    )
    # direct part: CR[i, NCORES*wspan + k*cspan + b*8 + j], -inf for non-direct
    cpart = cand[:, rt_of, p_of, :].transpose(1, 0, 2)          # [512, 8, 64]
    dmask = np.zeros((N, cspan), dtype=bool)
    for rt in range(RT):
        rsel = rt_of == rt
        for b in range(C):
            if kind[(rt, b)] == "direct":
                dmask[rsel, b * 8 : b * 8 + 8] = True
    CR[:, NCORES * wspan :] = np.where(
        dmask[:, None, :], cpart, -np.inf
    ).reshape(N, NCORES * cspan)

    # ---- remove positives by eps value matching ----
    owner = labels_p // L_LOCAL
    lloc = labels_p % L_LOCAL
    win = lloc // 8
    EPS = 1.5e-2
    for i in rows:
        rt = rt_of[i]
        k0 = owner[i]
        for c in range(C):
            if kind[(rt, c)] == "direct":
                idxs = np.arange(NCORES * wspan + k0 * cspan + c * 8,
                                 NCORES * wspan + k0 * cspan + c * 8 + 8)
                vals = CR[i, idxs]
                j = int(np.argmin(np.abs(vals - pos[i, c])))
                if abs(vals[j] - pos[i, c]) < EPS:
                    CR[i, idxs[j]] = -np.inf
            else:
                jj = k0 * wspan + c * NW + win[i]
                if abs(CR[i, jj] - pos[i, c]) < EPS:
                    CR[i, jj] = -np.inf

    part = np.partition(CR, CR.shape[1] - K, axis=1)[:, -K:]
    t50 = part.min(axis=1)

    # ---- certificate on direct blocks: 8th value must be <= t50 ----
    if _FORCE_FALLBACK:
        bad = rows
    else:
        worst = np.full(N, -np.inf)
        for rt in range(RT):
            rsel = rt_of == rt
            for b in range(C):
                if kind[(rt, b)] != "direct":
                    continue
                worst[rsel] = np.maximum(
                    worst[rsel],
                    cand[:, rt, :, b * 8 + 7].max(axis=0)[p_of[rsel]],
                )
        bad = np.where(worst > t50)[0]
    global _LAST_FALLBACKS
    _LAST_FALLBACKS = len(bad)
    for i in bad:
        sims_row = cen @ fn[i]                                  # [64000] exact
        sims_row[C * labels_p[i] : C * labels_p[i] + C] = -np.inf
        part[i] = np.sort(sims_row)[-K:]

    z = np.concatenate([pos, part], axis=1) * INV_T             # [512, 58]
    mz = z.max(axis=1)
    lse_inter = np.log(np.exp(z - mz[:, None]).sum(axis=1)) + mz
    loss_inter_i = lse_inter - INV_T * pos.mean(axis=1)

    # ---- per-camera means, summed ----
    cnt = np.bincount(cams_p, minlength=C).astype(np.float64)
    s_intra = np.bincount(cams_p, weights=loss_intra_i, minlength=C)
    s_inter = np.bincount(cams_p, weights=loss_inter_i, minlength=C)
    safe = np.maximum(cnt, 1.0)
    li = np.sum(np.where(cnt > 0, s_intra / safe, 0.0))
    le = LW * np.sum(np.where(cnt > 0, s_inter / safe, 0.0))
    return np.array([li, le], dtype=np.float32)


def _prepare(feats, indexes, label_table, cam_table, centers):
    feats = np.asarray(feats, dtype=np.float32)
    indexes = np.asarray(indexes)
    label_table = np.asarray(label_table)
    cam_table = np.asarray(cam_table)
    centers = np.asarray(centers, dtype=np.float32)

    labels = np.asarray(label_table[indexes], dtype=np.int64)
    cams = np.asarray(cam_table[indexes], dtype=np.int64)

    # permute rows so camera groups are contiguous, ordered big+small so most
    # 128-row tiles span only ~2 cameras (fewer intra exp instructions)
    sizes = np.bincount(cams, minlength=C)
    order = _pair_order(sizes)
    perm = np.concatenate([np.where(cams == c)[0] for c in order])
    fp = feats[perm].astype(np.float64)
    fp = fp / np.linalg.norm(fp, axis=1, keepdims=True)
    feats_p = np.ascontiguousarray(fp, dtype=np.float32)
    labels_p = labels[perm]
    cams_p = cams[perm]
    tile_cams = tuple(
        tuple(dict.fromkeys(cams_p[128 * rt : 128 * (rt + 1)].tolist()))
        for rt in range(RT)
    )
    # camera-major padded center shards: [2, 128, 8192] per core
    cenT_shards = []
    for k in range(NCORES):
        ck = centers[k * L_LOCAL * C : (k + 1) * L_LOCAL * C]
        ck = ck.reshape(L_LOCAL, C, D).transpose(1, 0, 2)   # [C, 1000, 256]
        pad = np.zeros((C, BPAD - L_LOCAL, D), dtype=np.float32)
        ckp = np.concatenate([ck, pad], axis=1)             # [C, 1024, 256]
        cenT = ckp.reshape(P_PAD, D).T                      # [256, 8192]
        cenT_shards.append(
            np.ascontiguousarray(cenT.reshape(2, 128, P_PAD), dtype=MM_NP)
        )
    return centers, tile_cams, feats_p, labels_p, cams_p, cenT_shards


def kernel(feats, indexes, label_table, cam_table, centers):
    centers, tile_cams, feats_p, labels_p, cams_p, cenT_shards = _prepare(
        feats, indexes, label_table, cam_table, centers
    )
    nc = _build_program(tile_cams)
    runner = _get_runner(nc)
    runner.put_inputs(_make_in_maps(cenT_shards, feats_p))
    results = runner.execute()
    return _host_finish(results, feats_p, labels_p, cams_p, centers, tile_cams)


# revision 40
# speedup vs baseline: 1.1864x; 1.0572x over previous
"""Trainium2 Bass kernel for nn_CAPMemory (camera-aware proxy memory loss).

Strategy (8 NeuronCores, SPMD, no collectives):
  - Shard the 64000x256 proxy table over labels: core k owns labels
    [1000k, 1000(k+1)), all 8 cameras.  On the host the shard is laid out
    CAMERA-MAJOR with each camera block padded 1000 -> 1024 columns
    (pad centers = 0 vectors): col c*1024 + l holds proxy (label l, cam c).
    This aligns camera blocks with PSUM banks and makes the intra-camera
    softmax read contiguous.
  - Feats are normalized and transposed on the host; the device runs a pure
    pipeline: DMA -> fp8e4 DoubleRow matmul (both 128-contraction halves in
    one instruction) -> per-1024-column-unit PSUM drain -> small outputs.
  - Each (row-tile, camera-block) unit [128 x 1024] in PSUM is drained by
    exactly one engine:
      direct unit: DVE MAX8 straight from PSUM -> top-8 values (f32)
      exp unit   : ACT Exp(scale=20) from PSUM -> bf16 exp values in SBUF
                   (+ accumulated per-camera exp-sum for the intra loss)
      window unit: ACT Copy from PSUM -> bf16 sims in SBUF
    bf16 units then go through a DVE pairwise-max tree over stride-125
    comb windows (all rounds in the 2x perf mode) to 8-element window maxes
    [128 x 125] shipped to the host (exp-domain values for exp units; the
    host takes log/20).
  - Host merge: intra logsumexp = log(sum_k srow_k); inter top-50 hard
    negatives merged from per-block top-8s and window maxes, positives
    removed by eps value matching; positives themselves recomputed exactly
    on host in f64.
  - Certificate on direct blocks (8th value <= merged t50) triggers exact
    per-row host recomputation; window blocks are statistically covered
    (window collisions lose at most one near-cutoff negative, effect on the
    loss ~1e-4 relative, validated offline against the reference).
"""

import sys
import functools

sys.path.insert(0, "/opt/trn_rl_repo")

import numpy as np
import ml_dtypes

from concourse import bacc, mybir
from concourse.tile import TileContext

F32 = mybir.dt.float32
BF16 = mybir.dt.bfloat16

N = 512          # batch
D = 256          # feature dim
L = 8000         # labels
C = 8            # cameras
NCORES = 8
RT = 4           # row tiles of 128
L_LOCAL = 1000   # labels per core
BPAD = 1024      # padded camera-block width
P_PAD = C * BPAD  # 8192 padded columns per core
INV_T = 20.0     # 1 / temperature
K = 50           # hard negatives
LW = 0.5         # inter-cam loss weight
NW = L_LOCAL // 8  # 125 8-wide windows per camera block

# total units drained by DVE MAX8 straight from PSUM (top-8 candidates);
# the rest are drained by ACT (exp or copy) and window-max-treed on DVE.
# ACT and DVE are the only engines that can read PSUM, so this splits the
# [512 x 8192] drain across both.
DIRECT_TOTAL = 10

# matmul operand dtype: float8e4 with the DoubleRow perf mode -- both
# 128-contraction halves fold into ONE matmul at 0.5 cyc/output-col (4x the
# bf16/f32r PE rate) and the centers DMA shrinks to 2.1 MB.  Quantization
# noise on the cosine sims is ~3e-3 RMS; end-to-end loss error validated at
# ~6e-4 relative (tolerance 2e-2).  Positives are recomputed exactly on the
# host, and the eps value-matching margins below absorb the noise.
MM_DT = mybir.dt.float8e4
MM_NP = ml_dtypes.float8_e4m3


def _pair_order(sizes):
    """Order cameras big+small so most 128-row tiles span only ~2 cameras."""
    desc = np.argsort(-np.asarray(sizes), kind="stable")
    big, small = desc[: C // 2], desc[C // 2 :][::-1]
    order = []
    for b, s in zip(big, small):
        order += [int(b), int(s)]
    return order


def _unit_plan(tile_cams):
    """Static drain plan: units in b-major order, kind per unit, tree pairs.

    Returns (units, kind, pairs, pair_of, wm_slot): pairs maps a pair id to
    its member units (1 or 2, same row tile); pair_of gives each tree unit
    its (pair id, member slot); wm_slot gives each tree unit its slot in the
    row tile's wm output (pair-order, so tree last rounds are contiguous).
    """
    units = [(rt, b) for b in range(C) for rt in range(RT)]
    kind = {}
    # distribute DIRECT_TOTAL direct units across row tiles (more directs to
    # tiles with fewer exp units)
    nexp = [len(tile_cams[rt]) for rt in range(RT)]
    d_rt = [0] * RT
    for _ in range(DIRECT_TOTAL):
        # give the next direct to the tile with most unassigned non-exp blocks
        loads = [(C - nexp[rt] - d_rt[rt], -rt) for rt in range(RT)]
        rt = -max(loads)[1]
        d_rt[rt] += 1
    # place the direct units so every b-column of the (b-major) drain order
    # gets ~one DVE-drained unit (rotating the row tile): ACT and DVE then
    # stay concurrently busy instead of alternating in bursts
    quota = list(d_rt)
    directs = set()
    for b in range(C):
        cands = [rt for rt in range(RT)
                 if b not in tile_cams[rt] and quota[rt] > 0]
        if not cands:
            continue
        rt = sorted(cands, key=lambda r: (-quota[r], (r - b) % RT))[0]
        directs.add((rt, b))
        quota[rt] -= 1
    for rt in range(RT):
        while quota[rt] > 0:
            ncol = {b: sum(1 for (r2, b2) in directs if b2 == b)
                    for b in range(C)}
            free = [b for b in range(C)
                    if b not in tile_cams[rt] and (rt, b) not in directs]
            b = sorted(free, key=lambda b2: (ncol[b2], b2))[0]
            directs.add((rt, b))
            quota[rt] -= 1
    for rt in range(RT):
        for b in range(C):
            if b in tile_cams[rt]:
                kind[(rt, b)] = "exp"
            elif (rt, b) in directs:
                kind[(rt, b)] = "direct"
            else:
                kind[(rt, b)] = "win"
    # pair tree units (exp+win) within each row tile in b order; a tree
    # unit's wm_rt slot is its position in that order, so every pair's last
    # round writes one contiguous wm_rt slice regardless of block adjacency
    pair_of = {}
    pairs = []
    wm_slot = {}
    for rt in range(RT):
        tus = [(rt, b) for b in range(C) if kind[(rt, b)] != "direct"]
        for s, u in enumerate(tus):
            wm_slot[u] = s
        for i in range(0, len(tus), 2):
            members = tus[i : i + 2]
            pid = len(pairs)
            pairs.append(members)
            for s, u in enumerate(members):
                pair_of[u] = (pid, s)
    return units, kind, pairs, pair_of, wm_slot


@functools.lru_cache(maxsize=8)
def _build_program(tile_cams, repeats=1):
    nc = bacc.Bacc(None, target_bir_lowering=False, num_swdge_queues=4)

    cenTd = nc.dram_tensor("cenT", [2, 128, P_PAD], MM_DT, kind="ExternalInput")
    fTd = nc.dram_tensor("fT", [128, RT, 2, 128], MM_DT, kind="ExternalInput")
    candd = nc.dram_tensor("cand", [RT, 128, C * 8], F32, kind="ExternalOutput")
    srowd = nc.dram_tensor("srow", [RT, 128, C], F32, kind="ExternalOutput")
    wmaxd = nc.dram_tensor("wmax", [RT, 128, C, NW], BF16, kind="ExternalOutput")

    with TileContext(nc) as tc:
        with (
            tc.tile_pool(name="cen", bufs=2) as cenp,
            tc.tile_pool(name="ftp", bufs=2) as ftp,
            tc.tile_pool(name="scrp", bufs=5) as scrp,
            tc.tile_pool(name="treep", bufs=3) as treep,
            tc.tile_pool(name="outp", bufs=2) as outp,
            tc.tile_pool(name="psum", bufs=4, space="PSUM") as psump,
        ):
            for _rep in range(repeats):
                _kernel_body(nc, tc, cenp, ftp, scrp, treep, outp, psump,
                             cenTd, fTd, candd, srowd, wmaxd, tile_cams)

    nc.compile()
    return nc


def _kernel_body(nc, tc, cenp, ftp, scrp, treep, outp, psump,
                 cenTd, fTd, candd, srowd, wmaxd, tile_cams):
    ActF = mybir.ActivationFunctionType

    units, kind, pairs, pair_of, wm_slot = _unit_plan(tile_cams)

    # ---- input DMA: fT first (matmuls need it immediately), then centers in
    # (h, block) granularity so early units unblock fast; alternate issuing
    # engines to spread descriptor generation across queues
    # inputs on SP only (so the next repeat's loads never queue behind this
    # repeat's output descriptors); outputs on gpsimd only
    fT_sb = ftp.tile([128, RT, 2, 128], MM_DT, name="fT_sb")
    nc.sync.dma_start(out=fT_sb[:, :, :, :], in_=fTd[:, :, :, :])
    cen_sb = cenp.tile([128, 2, P_PAD], MM_DT, name="cen_sb")
    for b in range(C):
        sl = slice(b * BPAD, (b + 1) * BPAD)
        for h in range(2):
            nc.sync.dma_start(out=cen_sb[:, h, sl], in_=cenTd[h, :, sl])

    cand_sb = [outp.tile([128, C * 8], F32, name=f"cand{rt}", bufs=2)
               for rt in range(RT)]
    s_t = [outp.tile([128, C], F32, name=f"st{rt}", bufs=2)
           for rt in range(RT)]
    wm_rt = [outp.tile([128, C, NW], BF16, name=f"wm{rt}", bufs=2)
             for rt in range(RT)]

    # pair state: scr tiles allocated lazily, members drain at different times
    pair_scr = [None] * len(pairs)
    pair_filled = [0] * len(pairs)
    # how many direct/exp/tree units remain per rt (to time the output DMAs)
    left_direct = [sum(1 for b in range(C) if kind[(rt, b)] == "direct")
                   for rt in range(RT)]
    left_exp = [len(tile_cams[rt]) for rt in range(RT)]
    left_tree = [sum(1 for b in range(C) if kind[(rt, b)] != "direct")
                 for rt in range(RT)]

    for ui, (rt, b) in enumerate(units):
        ps = psump.tile([128, BPAD], F32, name="ps")
        c0 = b * BPAD
        for j in range(2):
            nc.tensor.matmul(
                ps[:, j * 512 : (j + 1) * 512],
                fT_sb[:, rt, :, :],
                cen_sb[:, :, c0 + j * 512 : c0 + (j + 1) * 512],
                start=True, stop=True,
                perf_mode=mybir.MatmulPerfMode.DoubleRow,
            )
        k = kind[(rt, b)]
        if k == "direct":
            nc.vector.max(cand_sb[rt][:, b * 8 : b * 8 + 8], ps[:, 0:L_LOCAL])
            left_direct[rt] -= 1
            if left_direct[rt] == 0:
                nc.gpsimd.dma_start(out=candd[rt], in_=cand_sb[rt][:, :])
            continue
        pid, slot = pair_of[(rt, b)]
        npair = len(pairs[pid])
        if pair_scr[pid] is None:
            pair_scr[pid] = scrp.tile([128, npair, L_LOCAL], BF16, name="scr")
        scr = pair_scr[pid]
        if k == "exp":
            idx = tile_cams[rt].index(b)
            nc.scalar.activation(
                scr[:, slot, :], ps[:, 0:L_LOCAL], ActF.Exp,
                scale=INV_T, accum_out=s_t[rt][:, idx : idx + 1],
            )
            left_exp[rt] -= 1
            if left_exp[rt] == 0:
                nc.gpsimd.dma_start(out=srowd[rt], in_=s_t[rt][:, :])
        else:
            nc.scalar.copy(scr[:, slot, :], ps[:, 0:L_LOCAL])
        pair_filled[pid] += 1
        if pair_filled[pid] == npair:
            # bf16 pairwise-max tree over stride-125 comb windows (window w
            # holds local labels {w, 125+w, ..., 875+w}): the [8, 125] view
            # keeps every round's last dim packed and >=2 wide, so all three
            # rounds run in the DVE 2x perf mode
            v = scr.rearrange("p np (w nw) -> p np w nw", nw=NW)
            t1 = treep.tile([128, npair, 4, NW], BF16, name="t1")
            t2 = treep.tile([128, npair, 2, NW], BF16, name="t2")
            nc.vector.tensor_max(t1[:, :, :, :], v[:, :, 0:4, :],
                                 v[:, :, 4:8, :])
            nc.vector.tensor_max(t2[:, :, :, :], t1[:, :, 0:2, :],
                                 t1[:, :, 2:4, :])
            s0 = wm_slot[pairs[pid][0]]
            nc.vector.tensor_max(wm_rt[rt][:, s0 : s0 + npair, :],
                                 t2[:, :, 0, :], t2[:, :, 1, :])
            left_tree[rt] -= npair
            if left_tree[rt] == 0:
                nc.gpsimd.dma_start(out=wmaxd[rt], in_=wm_rt[rt][:, :, :])


class _Runner:
    """Sharded 8-core executor for a built Bass program.

    Builds the jax.jit(shard_map(bass_exec)) executable once (the walrus/NEFF
    compile happens inside the first call) and reuses it for every subsequent
    execution, keeping large inputs device-resident.
    """

    def __init__(self, nc, n_cores=NCORES):
        import jax
        from jax.sharding import Mesh, PartitionSpec, NamedSharding
        from jax.experimental.shard_map import shard_map
        from concourse import bass2jax

        self.jax = jax
        self.nc = nc
        self.n_cores = n_cores
        bass2jax.install_neuronx_cc_hook()
        partition_name = (
            nc.partition_id_tensor.name if nc.partition_id_tensor else None
        )
        in_names, out_names, out_avals = [], [], []
        for alloc in nc.m.functions[0].allocations:
            if not isinstance(alloc, mybir.MemoryLocationSet):
                continue
            name = alloc.memorylocations[0].name
            if alloc.kind == "ExternalInput":
                if name != partition_name:
                    in_names.append(name)
            elif alloc.kind == "ExternalOutput":
                out_names.append(name)
                out_avals.append(
                    jax.core.ShapedArray(
                        tuple(alloc.tensor_shape), mybir.dt.np(alloc.dtype)
                    )
                )
        self.in_names, self.out_names, self.out_avals = in_names, out_names, out_avals
        n_params, n_outs = len(in_names), len(out_avals)
        all_in_names = list(in_names) + list(out_names)
        if partition_name is not None:
            all_in_names.append(partition_name)

        def _body(*args):
            operands = list(args)
            if partition_name is not None:
                operands.append(bass2jax.partition_id_tensor())
            return tuple(
                bass2jax._bass_exec_p.bind(
                    *operands,
                    out_avals=tuple(out_avals),
                    in_names=tuple(all_in_names),
                    out_names=tuple(out_names),
                    lowering_input_output_aliases=(),
                    sim_require_finite=True,
                    sim_require_nnan=True,
                    nc=nc,
                )
            )

        devices = jax.devices()[:n_cores]
        self.mesh = Mesh(np.asarray(devices), ("core",))
        self.sh = NamedSharding(self.mesh, PartitionSpec("core"))
        self.fn = jax.jit(
            shard_map(
                _body,
                mesh=self.mesh,
                in_specs=(PartitionSpec("core"),) * (n_params + n_outs),
                out_specs=(PartitionSpec("core"),) * n_outs,
                check_rep=False,
            ),
            donate_argnums=tuple(range(n_params, n_params + n_outs)),
            keep_unused=True,
        )
        self._zero_shapes = [
            ((n_cores * a.shape[0], *a.shape[1:]), a.dtype) for a in out_avals
        ]

    def put_inputs(self, in_maps):
        self.dev_in = [
            self.jax.device_put(
                np.concatenate([np.asarray(m[name]) for m in in_maps], axis=0),
                self.sh,
            )
            for name in self.in_names
        ]

    def _zeros(self):
        return [
            self.jax.device_put(np.zeros(s, d), self.sh)
            for s, d in self._zero_shapes
        ]

    def execute(self):
        outs = self.fn(*self.dev_in, *self._zeros())
        self.jax.block_until_ready(outs)
        return self.unpack(outs)

    def unpack(self, outs):
        return [
            {
                name: np.asarray(outs[i]).reshape(
                    self.n_cores, *self.out_avals[i].shape
                )[c]
                for i, name in enumerate(self.out_names)
            }
            for c in range(self.n_cores)
        ]


_RUNNERS = {}
_LAST_FALLBACKS = 0
_FORCE_FALLBACK = False  # test hook: exercise the exact host fallback path


def _get_runner(nc):
    r = _RUNNERS.get(id(nc))
    if r is None:
        r = _Runner(nc)
        _RUNNERS[id(nc)] = r
    return r


def _make_in_maps(cenT_shards, feats_p):
    # feats_p is the permuted, L2-normalized batch; device wants the
    # transposed layout [q, rt, h, r] with q the contraction partition
    fT = np.ascontiguousarray(
        feats_p.reshape(RT, 128, 2, 128).transpose(3, 0, 2, 1), dtype=MM_NP
    )
    return [
        {"cenT": np.ascontiguousarray(cenT_shards[k], dtype=MM_NP), "fT": fT}
        for k in range(NCORES)
    ]


def _host_finish(results, feats_p, labels_p, cams_p, centers, tile_cams):
    units, kind, pairs, pair_of, wm_slot = _unit_plan(tile_cams)
    rows = np.arange(N)
    rt_of = rows // 128
    p_of = rows % 128

    # ---- intra: sum over cores of per-camera exp sums ----
    slot = np.zeros(N, dtype=np.int64)
    for rt in range(RT):
        for idx, cam in enumerate(tile_cams[rt]):
            sel = slice(128 * rt, 128 * (rt + 1))
            slot[sel] = np.where(cams_p[sel] == cam, idx, slot[sel])
    s_k = np.stack(
        [
            results[k]["srow"].reshape(RT, 128, C)[rt_of, p_of, slot]
            for k in range(NCORES)
        ]
    ).astype(np.float64)  # [8, 512]: sum_l exp(20 * cos sims) per core

    fn = feats_p.astype(np.float64)
    fn = fn / np.linalg.norm(fn, axis=1, keepdims=True)
    cen = centers.astype(np.float64)
    gidx = labels_p[:, None] * C + np.arange(C)[None, :]        # [512, 8]
    pos = np.einsum("rcd,rd->rc", cen[gidx], fn)                # [512, 8] f64

    lse_intra = np.log(s_k.sum(axis=0))
    v = pos[rows, cams_p]
    loss_intra_i = lse_intra - INV_T * v

    # ---- inter: merge candidates ----
    # direct blocks contribute their top-8 values; tree blocks their 125
    # window maxes (exp blocks in exp domain: s = log(w)/20).
    cand = np.stack([results[k]["cand"] for k in range(NCORES)])  # [8,RT,128,64]
    wmraw = np.stack(
        [results[k]["wmax"].astype(np.float32) for k in range(NCORES)]
    )  # [8,RT,128,C,NW], slot-indexed per row tile

    # remap slots -> blocks, converting exp-domain window maxes back to sims
    # domain; direct blocks have no window data (-inf)
    wmax = np.full_like(wmraw, -np.inf)
    for rt in range(RT):
        for b in range(C):
            kd = kind[(rt, b)]
            if kd == "direct":
                continue
            w = wmraw[:, rt, :, wm_slot[(rt, b)], :]
            if kd == "exp":
                w = np.log(np.maximum(w, 1e-30)) / INV_T
            wmax[:, rt, :, b, :] = w

    wspan = C * NW
    cspan = C * 8
    span = wspan + cspan
    CR = np.empty((N, NCORES * span), dtype=np.float64)
    # window part: CR[i, k*span + b*NW + w]
    CR[:, : NCORES * wspan].reshape(N, NCORES, wspan)[:] = (
        wmax[:, rt_of, p_of, :, :].transpose(1, 0, 2, 3).reshape(N, NCORES, wspan)
    )
    # direct part: CR[i, NCORES*wspan + k*cspan + b*8 + j], -inf for non-direct
    cpart = cand[:, rt_of, p_of, :].transpose(1, 0, 2)          # [512, 8, 64]
    dmask = np.zeros((N, cspan), dtype=bool)
    for rt in range(RT):
        rsel = rt_of == rt
        for b in range(C):
            if kind[(rt, b)] == "direct":
                dmask[rsel, b * 8 : b * 8 + 8] = True
    CR[:, NCORES * wspan :] = np.where(
        dmask[:, None, :], cpart, -np.inf
    ).reshape(N, NCORES * cspan)

    # ---- remove positives by eps value matching ----
    owner = labels_p // L_LOCAL
    lloc = labels_p % L_LOCAL
    win = lloc % NW  # stride-125 comb windows
    EPS = 1.5e-2
    for i in rows:
        rt = rt_of[i]
        k0 = owner[i]
        for c in range(C):
            if kind[(rt, c)] == "direct":
                idxs = np.arange(NCORES * wspan + k0 * cspan + c * 8,
                                 NCORES * wspan + k0 * cspan + c * 8 + 8)
                vals = CR[i, idxs]
                j = int(np.argmin(np.abs(vals - pos[i, c])))
                if abs(vals[j] - pos[i, c]) < EPS:
                    CR[i, idxs[j]] = -np.inf
            else:
                jj = k0 * wspan + c * NW + win[i]
                if abs(CR[i, jj] - pos[i, c]) < EPS:
                    CR[i, jj] = -np.inf

    part = np.partition(CR, CR.shape[1] - K, axis=1)[:, -K:]
    t50 = part.min(axis=1)

    # ---- certificate on direct blocks: 8th value must be <= t50 ----
    if _FORCE_FALLBACK:
        bad = rows
    else:
        worst = np.full(N, -np.inf)
        for rt in range(RT):
            rsel = rt_of == rt
            for b in range(C):
                if kind[(rt, b)] != "direct":
                    continue
                worst[rsel] = np.maximum(
                    worst[rsel],
                    cand[:, rt, :, b * 8 + 7].max(axis=0)[p_of[rsel]],
                )
        bad = np.where(worst > t50)[0]
    global _LAST_FALLBACKS
    _LAST_FALLBACKS = len(bad)
    for i in bad:
        sims_row = cen @ fn[i]                                  # [64000] exact
        sims_row[C * labels_p[i] : C * labels_p[i] + C] = -np.inf
        part[i] = np.sort(sims_row)[-K:]

    z = np.concatenate([pos, part], axis=1) * INV_T             # [512, 58]
    mz = z.max(axis=1)
    lse_inter = np.log(np.exp(z - mz[:, None]).sum(axis=1)) + mz
    loss_inter_i = lse_inter - INV_T * pos.mean(axis=1)

    # ---- per-camera means, summed ----
    cnt = np.bincount(cams_p, minlength=C).astype(np.float64)
    s_intra = np.bincount(cams_p, weights=loss_intra_i, minlength=C)
    s_inter = np.bincount(cams_p, weights=loss_inter_i, minlength=C)
    safe = np.maximum(cnt, 1.0)
    li = np.sum(np.where(cnt > 0, s_intra / safe, 0.0))
    le = LW * np.sum(np.where(cnt > 0, s_inter / safe, 0.0))
    return np.array([li, le], dtype=np.float32)


def _prepare(feats, indexes, label_table, cam_table, centers):
    feats = np.asarray(feats, dtype=np.float32)
    indexes = np.asarray(indexes)
    label_table = np.asarray(label_table)
    cam_table = np.asarray(cam_table)
    centers = np.asarray(centers, dtype=np.float32)

    labels = np.asarray(label_table[indexes], dtype=np.int64)
    cams = np.asarray(cam_table[indexes], dtype=np.int64)

    # permute rows so camera groups are contiguous, ordered big+small so most
    # 128-row tiles span only ~2 cameras (fewer intra exp instructions)
    sizes = np.bincount(cams, minlength=C)
    order = _pair_order(sizes)
    perm = np.concatenate([np.where(cams == c)[0] for c in order])
    fp = feats[perm].astype(np.float64)
    fp = fp / np.linalg.norm(fp, axis=1, keepdims=True)
    feats_p = np.ascontiguousarray(fp, dtype=np.float32)
    labels_p = labels[perm]
    cams_p = cams[perm]
    tile_cams = tuple(
        tuple(dict.fromkeys(cams_p[128 * rt : 128 * (rt + 1)].tolist()))
        for rt in range(RT)
    )
    # camera-major padded center shards: [2, 128, 8192] per core
    cenT_shards = []
    for k in range(NCORES):
        ck = centers[k * L_LOCAL * C : (k + 1) * L_LOCAL * C]
        ck = ck.reshape(L_LOCAL, C, D).transpose(1, 0, 2)   # [C, 1000, 256]
        pad = np.zeros((C, BPAD - L_LOCAL, D), dtype=np.float32)
        ckp = np.concatenate([ck, pad], axis=1)             # [C, 1024, 256]
        cenT = ckp.reshape(P_PAD, D).T                      # [256, 8192]
        cenT_shards.append(
            np.ascontiguousarray(cenT.reshape(2, 128, P_PAD), dtype=MM_NP)
        )
    return centers, tile_cams, feats_p, labels_p, cams_p, cenT_shards


def kernel(feats, indexes, label_table, cam_table, centers):
    centers, tile_cams, feats_p, labels_p, cams_p, cenT_shards = _prepare(
        feats, indexes, label_table, cam_table, centers
    )
    nc = _build_program(tile_cams)
    runner = _get_runner(nc)
    runner.put_inputs(_make_in_maps(cenT_shards, feats_p))
    results = runner.execute()
    return _host_finish(results, feats_p, labels_p, cams_p, centers, tile_cams)


# revision 48
# speedup vs baseline: 1.1868x; 1.0004x over previous
"""Trainium2 Bass kernel for nn_CAPMemory (camera-aware proxy memory loss).

Strategy (8 NeuronCores, SPMD, no collectives):
  - Shard the 64000x256 proxy table over labels: core k owns labels
    [1000k, 1000(k+1)), all 8 cameras.  On the host the shard is laid out
    CAMERA-MAJOR with each camera block padded 1000 -> 1024 columns
    (pad centers = 0 vectors): col c*1024 + l holds proxy (label l, cam c).
    This aligns camera blocks with PSUM banks and makes the intra-camera
    softmax read contiguous.
  - Feats are normalized and transposed on the host; the device runs a pure
    pipeline: DMA -> fp8e4 DoubleRow matmul (both 128-contraction halves in
    one instruction) -> per-1024-column-unit PSUM drain -> small outputs.
  - Each (row-tile, camera-block) unit [128 x 1024] in PSUM is drained by
    exactly one engine:
      direct unit: DVE MAX8 straight from PSUM -> top-8 values (f32)
      exp unit   : ACT Exp(scale=20) from PSUM -> bf16 exp values in SBUF
                   (+ accumulated per-camera exp-sum for the intra loss)
      window unit: ACT Copy from PSUM -> bf16 sims in SBUF
    bf16 units then go through a DVE pairwise-max tree over stride-125
    comb windows (all rounds in the 2x perf mode) to 8-element window maxes
    [128 x 125] shipped to the host (exp-domain values for exp units; the
    host takes log/20).
  - Host merge: intra logsumexp = log(sum_k srow_k); inter top-50 hard
    negatives merged from per-block top-8s and window maxes, positives
    removed by eps value matching; positives themselves recomputed exactly
    on host in f64.
  - Certificate on direct blocks (8th value <= merged t50) triggers exact
    per-row host recomputation; window blocks are statistically covered
    (window collisions lose at most one near-cutoff negative, effect on the
    loss ~1e-4 relative, validated offline against the reference).
"""

import sys
import functools

sys.path.insert(0, "/opt/trn_rl_repo")

import numpy as np
import ml_dtypes

from concourse import bacc, mybir
from concourse.tile import TileContext

F32 = mybir.dt.float32
BF16 = mybir.dt.bfloat16

N = 512          # batch
D = 256          # feature dim
L = 8000         # labels
C = 8            # cameras
NCORES = 8
RT = 4           # row tiles of 128
L_LOCAL = 1000   # labels per core
BPAD = 1024      # padded camera-block width
P_PAD = C * BPAD  # 8192 padded columns per core
INV_T = 20.0     # 1 / temperature
K = 50           # hard negatives
LW = 0.5         # inter-cam loss weight
NW = L_LOCAL // 8  # 125 8-wide windows per camera block

# total units drained by DVE MAX8 straight from PSUM (top-8 candidates);
# the rest are drained by ACT (exp or copy) and window-max-treed on DVE.
# ACT and DVE are the only engines that can read PSUM, so this splits the
# [512 x 8192] drain across both.
DIRECT_TOTAL = 10

# matmul operand dtype: float8e4 with the DoubleRow perf mode -- both
# 128-contraction halves fold into ONE matmul at 0.5 cyc/output-col (4x the
# bf16/f32r PE rate) and the centers DMA shrinks to 2.1 MB.  Quantization
# noise on the cosine sims is ~3e-3 RMS; end-to-end loss error validated at
# ~6e-4 relative (tolerance 2e-2).  Positives are recomputed exactly on the
# host, and the eps value-matching margins below absorb the noise.
MM_DT = mybir.dt.float8e4
MM_NP = ml_dtypes.float8_e4m3


def _pair_order(sizes):
    """Order cameras big+small so most 128-row tiles span only ~2 cameras."""
    desc = np.argsort(-np.asarray(sizes), kind="stable")
    big, small = desc[: C // 2], desc[C // 2 :][::-1]
    order = []
    for b, s in zip(big, small):
        order += [int(b), int(s)]
    return order


def _unit_plan(tile_cams):
    """Static drain plan: units in b-major order, kind per unit, tree pairs.

    Returns (units, kind, pairs, pair_of, wm_slot): pairs maps a pair id to
    its member units (1 or 2, same row tile); pair_of gives each tree unit
    its (pair id, member slot); wm_slot gives each tree unit its slot in the
    row tile's wm output (pair-order, so tree last rounds are contiguous).
    """
    units = [(rt, b) for b in range(C) for rt in range(RT)]
    kind = {}
    # distribute DIRECT_TOTAL direct units across row tiles (more directs to
    # tiles with fewer exp units)
    nexp = [len(tile_cams[rt]) for rt in range(RT)]
    d_rt = [0] * RT
    for _ in range(DIRECT_TOTAL):
        # give the next direct to the tile with most unassigned non-exp blocks
        loads = [(C - nexp[rt] - d_rt[rt], -rt) for rt in range(RT)]
        rt = -max(loads)[1]
        d_rt[rt] += 1
    # place the direct units so every b-column of the (b-major) drain order
    # gets ~one DVE-drained unit (rotating the row tile): ACT and DVE then
    # stay concurrently busy instead of alternating in bursts
    quota = list(d_rt)
    directs = set()
    for b in range(C):
        cands = [rt for rt in range(RT)
                 if b not in tile_cams[rt] and quota[rt] > 0]
        if not cands:
            continue
        rt = sorted(cands, key=lambda r: (-quota[r], (r - b) % RT))[0]
        directs.add((rt, b))
        quota[rt] -= 1
    for rt in range(RT):
        while quota[rt] > 0:
            ncol = {b: sum(1 for (r2, b2) in directs if b2 == b)
                    for b in range(C)}
            free = [b for b in range(C)
                    if b not in tile_cams[rt] and (rt, b) not in directs]
            b = sorted(free, key=lambda b2: (ncol[b2], b2))[0]
            directs.add((rt, b))
            quota[rt] -= 1
    for rt in range(RT):
        for b in range(C):
            if b in tile_cams[rt]:
                kind[(rt, b)] = "exp"
            elif (rt, b) in directs:
                kind[(rt, b)] = "direct"
            else:
                kind[(rt, b)] = "win"
    # pair tree units (exp+win) within each row tile in b order; a tree
    # unit's wm_rt slot is its position in that order, so every pair's last
    # round writes one contiguous wm_rt slice regardless of block adjacency
    pair_of = {}
    pairs = []
    wm_slot = {}
    for rt in range(RT):
        tus = [(rt, b) for b in range(C) if kind[(rt, b)] != "direct"]
        for s, u in enumerate(tus):
            wm_slot[u] = s
        for i in range(0, len(tus), 2):
            members = tus[i : i + 2]
            pid = len(pairs)
            pairs.append(members)
            for s, u in enumerate(members):
                pair_of[u] = (pid, s)
    return units, kind, pairs, pair_of, wm_slot


@functools.lru_cache(maxsize=8)
def _build_program(tile_cams, repeats=1):
    nc = bacc.Bacc(None, target_bir_lowering=False, num_swdge_queues=4)

    cenTd = nc.dram_tensor("cenT", [2, 128, P_PAD], MM_DT, kind="ExternalInput")
    fTd = nc.dram_tensor("fT", [128, RT, 2, 128], MM_DT, kind="ExternalInput")
    candd = nc.dram_tensor("cand", [RT, 128, C * 8], F32, kind="ExternalOutput")
    srowd = nc.dram_tensor("srow", [RT, 128, C], F32, kind="ExternalOutput")
    wmaxd = nc.dram_tensor("wmax", [RT, 128, C, NW], BF16, kind="ExternalOutput")

    with TileContext(nc) as tc:
        with (
            tc.tile_pool(name="cen", bufs=2) as cenp,
            tc.tile_pool(name="ftp", bufs=2) as ftp,
            tc.tile_pool(name="scrp", bufs=5) as scrp,
            tc.tile_pool(name="treep", bufs=3) as treep,
            tc.tile_pool(name="outp", bufs=2) as outp,
            tc.tile_pool(name="psum", bufs=4, space="PSUM") as psump,
        ):
            for _rep in range(repeats):
                _kernel_body(nc, tc, cenp, ftp, scrp, treep, outp, psump,
                             cenTd, fTd, candd, srowd, wmaxd, tile_cams)

    nc.compile()
    return nc


def _kernel_body(nc, tc, cenp, ftp, scrp, treep, outp, psump,
                 cenTd, fTd, candd, srowd, wmaxd, tile_cams):
    ActF = mybir.ActivationFunctionType

    units, kind, pairs, pair_of, wm_slot = _unit_plan(tile_cams)

    # ---- input DMA: fT first (matmuls need it immediately), then centers in
    # (h, block) granularity so early units unblock fast; alternate issuing
    # engines to spread descriptor generation across queues
    # inputs on SP only (so the next repeat's loads never queue behind this
    # repeat's output descriptors); outputs on gpsimd only
    fT_sb = ftp.tile([128, RT, 2, 128], MM_DT, name="fT_sb")
    nc.sync.dma_start(out=fT_sb[:, :, :, :], in_=fTd[:, :, :, :])
    cen_sb = cenp.tile([128, 2, P_PAD], MM_DT, name="cen_sb")
    for b in range(C):
        sl = slice(b * BPAD, (b + 1) * BPAD)
        for h in range(2):
            nc.sync.dma_start(out=cen_sb[:, h, sl], in_=cenTd[h, :, sl])

    cand_sb = [outp.tile([128, C * 8], F32, name=f"cand{rt}", bufs=2)
               for rt in range(RT)]
    s_t = [outp.tile([128, C], F32, name=f"st{rt}", bufs=2)
           for rt in range(RT)]
    wm_rt = [outp.tile([128, C, NW], BF16, name=f"wm{rt}", bufs=2)
             for rt in range(RT)]

    # pair state: scr tiles allocated lazily, members drain at different times
    pair_scr = [None] * len(pairs)
    pair_filled = [0] * len(pairs)
    # how many direct/exp/tree units remain per rt (to time the output DMAs)
    left_direct = [sum(1 for b in range(C) if kind[(rt, b)] == "direct")
                   for rt in range(RT)]
    left_exp = [len(tile_cams[rt]) for rt in range(RT)]
    left_tree = [sum(1 for b in range(C) if kind[(rt, b)] != "direct")
                 for rt in range(RT)]

    for ui, (rt, b) in enumerate(units):
        ps = psump.tile([128, BPAD], F32, name="ps")
        c0 = b * BPAD
        for j in range(2):
            nc.tensor.matmul(
                ps[:, j * 512 : (j + 1) * 512],
                fT_sb[:, rt, :, :],
                cen_sb[:, :, c0 + j * 512 : c0 + (j + 1) * 512],
                start=True, stop=True,
                perf_mode=mybir.MatmulPerfMode.DoubleRow,
            )
        k = kind[(rt, b)]
        if k == "direct":
            nc.vector.max(cand_sb[rt][:, b * 8 : b * 8 + 8], ps[:, 0:L_LOCAL])
            left_direct[rt] -= 1
            if left_direct[rt] == 0:
                nc.gpsimd.dma_start(out=candd[rt], in_=cand_sb[rt][:, :])
            continue
        pid, slot = pair_of[(rt, b)]
        npair = len(pairs[pid])
        if pair_scr[pid] is None:
            pair_scr[pid] = scrp.tile([128, npair, L_LOCAL], BF16, name="scr")
        scr = pair_scr[pid]
        if k == "exp":
            idx = tile_cams[rt].index(b)
            nc.scalar.activation(
                scr[:, slot, :], ps[:, 0:L_LOCAL], ActF.Exp,
                scale=INV_T, accum_out=s_t[rt][:, idx : idx + 1],
            )
            left_exp[rt] -= 1
            if left_exp[rt] == 0:
                nc.gpsimd.dma_start(out=srowd[rt], in_=s_t[rt][:, :])
        else:
            nc.scalar.copy(scr[:, slot, :], ps[:, 0:L_LOCAL])
        pair_filled[pid] += 1
        if pair_filled[pid] == npair:
            # bf16 pairwise-max tree over stride-125 comb windows (window w
            # holds local labels {w, 125+w, ..., 875+w}): the [8, 125] view
            # keeps every round's last dim packed and >=2 wide, so all three
            # rounds run in the DVE 2x perf mode
            v = scr.rearrange("p np (w nw) -> p np w nw", nw=NW)
            t1 = treep.tile([128, npair, 4, NW], BF16, name="t1")
            t2 = treep.tile([128, npair, 2, NW], BF16, name="t2")
            nc.vector.tensor_max(t1[:, :, :, :], v[:, :, 0:4, :],
                                 v[:, :, 4:8, :])
            nc.vector.tensor_max(t2[:, :, :, :], t1[:, :, 0:2, :],
                                 t1[:, :, 2:4, :])
            s0 = wm_slot[pairs[pid][0]]
            nc.vector.tensor_max(wm_rt[rt][:, s0 : s0 + npair, :],
                                 t2[:, :, 0, :], t2[:, :, 1, :])
            left_tree[rt] -= npair
            if left_tree[rt] == 0:
                nc.gpsimd.dma_start(out=wmaxd[rt], in_=wm_rt[rt][:, :, :])


class _Runner:
    """Sharded 8-core executor for a built Bass program.

    Builds the jax.jit(shard_map(bass_exec)) executable once (the walrus/NEFF
    compile happens inside the first call) and reuses it for every subsequent
    execution, keeping large inputs device-resident.
    """

    def __init__(self, nc, n_cores=NCORES):
        import jax
        from jax.sharding import Mesh, PartitionSpec, NamedSharding
        from jax.experimental.shard_map import shard_map
        from concourse import bass2jax

        self.jax = jax
        self.nc = nc
        self.n_cores = n_cores
        bass2jax.install_neuronx_cc_hook()
        partition_name = (
            nc.partition_id_tensor.name if nc.partition_id_tensor else None
        )
        in_names, out_names, out_avals = [], [], []
        for alloc in nc.m.functions[0].allocations:
            if not isinstance(alloc, mybir.MemoryLocationSet):
                continue
            name = alloc.memorylocations[0].name
            if alloc.kind == "ExternalInput":
                if name != partition_name:
                    in_names.append(name)
            elif alloc.kind == "ExternalOutput":
                out_names.append(name)
                out_avals.append(
                    jax.core.ShapedArray(
                        tuple(alloc.tensor_shape), mybir.dt.np(alloc.dtype)
                    )
                )
        self.in_names, self.out_names, self.out_avals = in_names, out_names, out_avals
        n_params, n_outs = len(in_names), len(out_avals)
        all_in_names = list(in_names) + list(out_names)
        if partition_name is not None:
            all_in_names.append(partition_name)

        def _body(*args):
            operands = list(args)
            if partition_name is not None:
                operands.append(bass2jax.partition_id_tensor())
            return tuple(
                bass2jax._bass_exec_p.bind(
                    *operands,
                    out_avals=tuple(out_avals),
                    in_names=tuple(all_in_names),
                    out_names=tuple(out_names),
                    lowering_input_output_aliases=(),
                    sim_require_finite=True,
                    sim_require_nnan=True,
                    nc=nc,
                )
            )

        devices = jax.devices()[:n_cores]
        self.mesh = Mesh(np.asarray(devices), ("core",))
        self.sh = NamedSharding(self.mesh, PartitionSpec("core"))
        self.fn = jax.jit(
            shard_map(
                _body,
                mesh=self.mesh,
                in_specs=(PartitionSpec("core"),) * (n_params + n_outs),
                out_specs=(PartitionSpec("core"),) * n_outs,
                check_rep=False,
            ),
            donate_argnums=tuple(range(n_params, n_params + n_outs)),
            keep_unused=True,
        )
        self._zero_shapes = [
            ((n_cores * a.shape[0], *a.shape[1:]), a.dtype) for a in out_avals
        ]

    def put_inputs(self, in_maps):
        self.dev_in = [
            self.jax.device_put(
                np.concatenate([np.asarray(m[name]) for m in in_maps], axis=0),
                self.sh,
            )
            for name in self.in_names
        ]

    def _zeros(self):
        return [
            self.jax.device_put(np.zeros(s, d), self.sh)
            for s, d in self._zero_shapes
        ]

    def execute(self):
        outs = self.fn(*self.dev_in, *self._zeros())
        self.jax.block_until_ready(outs)
        return self.unpack(outs)

    def unpack(self, outs):
        return [
            {
                name: np.asarray(outs[i]).reshape(
                    self.n_cores, *self.out_avals[i].shape
                )[c]
                for i, name in enumerate(self.out_names)
            }
            for c in range(self.n_cores)
        ]


_RUNNERS = {}
_LAST_FALLBACKS = 0
_FORCE_FALLBACK = False  # test hook: exercise the exact host fallback path


def _get_runner(nc):
    r = _RUNNERS.get(id(nc))
    if r is None:
        r = _Runner(nc)
        _RUNNERS[id(nc)] = r
    return r


def _make_in_maps(cenT_shards, feats_p):
    # feats_p is the permuted, L2-normalized batch; device wants the
    # transposed layout [q, rt, h, r] with q the contraction partition
    fT = np.ascontiguousarray(
        feats_p.reshape(RT, 128, 2, 128).transpose(3, 0, 2, 1), dtype=MM_NP
    )
    return [
        {"cenT": np.ascontiguousarray(cenT_shards[k], dtype=MM_NP), "fT": fT}
        for k in range(NCORES)
    ]


def _host_finish(results, feats_p, labels_p, cams_p, centers, tile_cams):
    units, kind, pairs, pair_of, wm_slot = _unit_plan(tile_cams)
    rows = np.arange(N)
    rt_of = rows // 128
    p_of = rows % 128

    # ---- intra: sum over cores of per-camera exp sums ----
    slot = np.zeros(N, dtype=np.int64)
    for rt in range(RT):
        for idx, cam in enumerate(tile_cams[rt]):
            sel = slice(128 * rt, 128 * (rt + 1))
            slot[sel] = np.where(cams_p[sel] == cam, idx, slot[sel])
    s_k = np.stack(
        [
            results[k]["srow"].reshape(RT, 128, C)[rt_of, p_of, slot]
            for k in range(NCORES)
        ]
    ).astype(np.float64)  # [8, 512]: sum_l exp(20 * cos sims) per core

    fn = feats_p.astype(np.float64)
    fn = fn / np.linalg.norm(fn, axis=1, keepdims=True)
    cen = centers.astype(np.float64)
    gidx = labels_p[:, None] * C + np.arange(C)[None, :]        # [512, 8]
    pos = np.einsum("rcd,rd->rc", cen[gidx], fn)                # [512, 8] f64

    lse_intra = np.log(s_k.sum(axis=0))
    v = pos[rows, cams_p]
    loss_intra_i = lse_intra - INV_T * v

    # ---- inter: merge candidates ----
    # direct blocks contribute their top-8 values; tree blocks their 125
    # window maxes (exp blocks in exp domain: s = log(w)/20).
    cand = np.stack([results[k]["cand"] for k in range(NCORES)])  # [8,RT,128,64]
    wmraw = np.stack(
        [results[k]["wmax"].astype(np.float32) for k in range(NCORES)]
    )  # [8,RT,128,C,NW], slot-indexed per row tile

    # remap slots -> blocks, converting exp-domain window maxes back to sims
    # domain; direct blocks have no window data (-inf)
    wmax = np.full_like(wmraw, -np.inf)
    for rt in range(RT):
        for b in range(C):
            kd = kind[(rt, b)]
            if kd == "direct":
                continue
            w = wmraw[:, rt, :, wm_slot[(rt, b)], :]
            if kd == "exp":
                w = np.log(np.maximum(w, 1e-30)) / INV_T
            wmax[:, rt, :, b, :] = w

    wspan = C * NW
    cspan = C * 8
    span = wspan + cspan
    CR = np.empty((N, NCORES * span), dtype=np.float64)
    # window part: CR[i, k*span + b*NW + w]
    CR[:, : NCORES * wspan].reshape(N, NCORES, wspan)[:] = (
        wmax[:, rt_of, p_of, :, :].transpose(1, 0, 2, 3).reshape(N, NCORES, wspan)
    )
    # direct part: CR[i, NCORES*wspan + k*cspan + b*8 + j], -inf for non-direct
    cpart = cand[:, rt_of, p_of, :].transpose(1, 0, 2)          # [512, 8, 64]
    dmask = np.zeros((N, cspan), dtype=bool)
    for rt in range(RT):
        rsel = rt_of == rt
        for b in range(C):
            if kind[(rt, b)] == "direct":
                dmask[rsel, b * 8 : b * 8 + 8] = True
    CR[:, NCORES * wspan :] = np.where(
        dmask[:, None, :], cpart, -np.inf
    ).reshape(N, NCORES * cspan)

    # ---- remove positives by eps value matching ----
    owner = labels_p // L_LOCAL
    lloc = labels_p % L_LOCAL
    win = lloc % NW  # stride-125 comb windows
    EPS = 1.5e-2
    for i in rows:
        rt = rt_of[i]
        k0 = owner[i]
        for c in range(C):
            if kind[(rt, c)] == "direct":
                idxs = np.arange(NCORES * wspan + k0 * cspan + c * 8,
                                 NCORES * wspan + k0 * cspan + c * 8 + 8)
                vals = CR[i, idxs]
                j = int(np.argmin(np.abs(vals - pos[i, c])))
                if abs(vals[j] - pos[i, c]) < EPS:
                    CR[i, idxs[j]] = -np.inf
            else:
                jj = k0 * wspan + c * NW + win[i]
                if abs(CR[i, jj] - pos[i, c]) < EPS:
                    CR[i, jj] = -np.inf

    part = np.partition(CR, CR.shape[1] - K, axis=1)[:, -K:]
    t50 = part.min(axis=1)

    # ---- certificate on direct blocks: 8th value must be <= t50 ----
    if _FORCE_FALLBACK:
        bad = rows
    else:
        worst = np.full(N, -np.inf)
        for rt in range(RT):
            rsel = rt_of == rt
            for b in range(C):
                if kind[(rt, b)] != "direct":
                    continue
                worst[rsel] = np.maximum(
                    worst[rsel],
                    cand[:, rt, :, b * 8 + 7].max(axis=0)[p_of[rsel]],
                )
        bad = np.where(worst > t50)[0]
    global _LAST_FALLBACKS
    _LAST_FALLBACKS = len(bad)
    for i in bad:
        sims_row = cen @ fn[i]                                  # [64000] exact
        sims_row[C * labels_p[i] : C * labels_p[i] + C] = -np.inf
        part[i] = np.sort(sims_row)[-K:]

    z = np.concatenate([pos, part], axis=1) * INV_T             # [512, 58]
    mz = z.max(axis=1)
    lse_inter = np.log(np.exp(z - mz[:, None]).sum(axis=1)) + mz
    loss_inter_i = lse_inter - INV_T * pos.mean(axis=1)

    # ---- per-camera means, summed ----
    cnt = np.bincount(cams_p, minlength=C).astype(np.float64)
    s_intra = np.bincount(cams_p, weights=loss_intra_i, minlength=C)
    s_inter = np.bincount(cams_p, weights=loss_inter_i, minlength=C)
    safe = np.maximum(cnt, 1.0)
    li = np.sum(np.where(cnt > 0, s_intra / safe, 0.0))
    le = LW * np.sum(np.where(cnt > 0, s_inter / safe, 0.0))
    return np.array([li, le], dtype=np.float32)


def _prepare(feats, indexes, label_table, cam_table, centers):
    feats = np.asarray(feats, dtype=np.float32)
    indexes = np.asarray(indexes)
    label_table = np.asarray(label_table)
    cam_table = np.asarray(cam_table)
    centers = np.asarray(centers, dtype=np.float32)

    labels = np.asarray(label_table[indexes], dtype=np.int64)
    cams = np.asarray(cam_table[indexes], dtype=np.int64)

    # permute rows so camera groups are contiguous, ordered big+small so most
    # 128-row tiles span only ~2 cameras (fewer intra exp instructions)
    sizes = np.bincount(cams, minlength=C)
    order = _pair_order(sizes)
    perm = np.concatenate([np.where(cams == c)[0] for c in order])
    fp = feats[perm].astype(np.float64)
    fp = fp / np.linalg.norm(fp, axis=1, keepdims=True)
    feats_p = np.ascontiguousarray(fp, dtype=np.float32)
    labels_p = labels[perm]
    cams_p = cams[perm]
    tile_cams = tuple(
        tuple(dict.fromkeys(cams_p[128 * rt : 128 * (rt + 1)].tolist()))
        for rt in range(RT)
    )
    # camera-major padded center shards: [2, 128, 8192] per core
    cenT_shards = []
    for k in range(NCORES):
        ck = centers[k * L_LOCAL * C : (k + 1) * L_LOCAL * C]
        ck = ck.reshape(L_LOCAL, C, D).transpose(1, 0, 2)   # [C, 1000, 256]
        pad = np.zeros((C, BPAD - L_LOCAL, D), dtype=np.float32)
        ckp = np.concatenate([ck, pad], axis=1)             # [C, 1024, 256]
        cenT = ckp.reshape(P_PAD, D).T                      # [256, 8192]
        cenT_shards.append(
            np.ascontiguousarray(cenT.reshape(2, 128, P_PAD), dtype=MM_NP)
        )
    return centers, tile_cams, feats_p, labels_p, cams_p, cenT_shards


def kernel(feats, indexes, label_table, cam_table, centers):
    centers, tile_cams, feats_p, labels_p, cams_p, cenT_shards = _prepare(
        feats, indexes, label_table, cam_table, centers
    )
    nc = _build_program(tile_cams)
    runner = _get_runner(nc)
    runner.put_inputs(_make_in_maps(cenT_shards, feats_p))
    results = runner.execute()
    return _host_finish(results, feats_p, labels_p, cams_p, centers, tile_cams)


# revision 49
# speedup vs baseline: 1.2580x; 1.0600x over previous
"""Trainium2 Bass kernel for nn_CAPMemory (camera-aware proxy memory loss).

Strategy (8 NeuronCores, SPMD, no collectives):
  - Shard the 64000x256 proxy table over labels: core k owns labels
    [1000k, 1000(k+1)), all 8 cameras.  On the host the shard is laid out
    CAMERA-MAJOR with each camera block padded 1000 -> 1024 columns
    (pad centers = 0 vectors): col c*1024 + l holds proxy (label l, cam c).
    This aligns camera blocks with PSUM banks and makes the intra-camera
    softmax read contiguous.
  - Feats are normalized and transposed on the host; the device runs a pure
    pipeline: DMA -> fp8e4 DoubleRow matmul (both 128-contraction halves in
    one instruction) -> per-1024-column-unit PSUM drain -> small outputs.
  - Each (row-tile, camera-block) unit [128 x 1024] in PSUM is drained by
    exactly one engine:
      direct unit: DVE MAX8 straight from PSUM -> top-8 values (f32)
      exp unit   : ACT Exp(scale=20) from PSUM -> bf16 exp values in SBUF
                   (+ accumulated per-camera exp-sum for the intra loss)
      window unit: ACT Copy from PSUM -> bf16 sims in SBUF
    bf16 units then go through a DVE pairwise-max tree over stride-125
    comb windows (all rounds in the 2x perf mode) to 8-element window maxes
    [128 x 125] shipped to the host (exp-domain values for exp units; the
    host takes log/20).
  - Host merge: intra logsumexp = log(sum_k srow_k); inter top-50 hard
    negatives merged from per-block top-8s and window maxes, positives
    removed by eps value matching; positives themselves recomputed exactly
    on host in f64.
  - Certificate on direct blocks (8th value <= merged t50) triggers exact
    per-row host recomputation; window blocks are statistically covered
    (window collisions lose at most one near-cutoff negative, effect on the
    loss ~1e-4 relative, validated offline against the reference).
"""

import sys
import functools

sys.path.insert(0, "/opt/trn_rl_repo")

import numpy as np
import ml_dtypes

from concourse import bacc, mybir
from concourse.tile import TileContext

F32 = mybir.dt.float32
BF16 = mybir.dt.bfloat16

N = 512          # batch
D = 256          # feature dim
L = 8000         # labels
C = 8            # cameras
NCORES = 8
RT = 4           # row tiles of 128
L_LOCAL = 1000   # labels per core
BPAD = 1024      # padded camera-block width
P_PAD = C * BPAD  # 8192 padded columns per core
INV_T = 20.0     # 1 / temperature
K = 50           # hard negatives
LW = 0.5         # inter-cam loss weight
NW = L_LOCAL // 2  # 500 2-wide comb windows per camera block

# total units drained by DVE MAX8 straight from PSUM (top-8 candidates);
# the rest are drained by ACT (exp or copy) and window-max-treed on DVE.
# ACT and DVE are the only engines that can read PSUM, so this splits the
# [512 x 8192] drain across both.
DIRECT_TOTAL = 12

# matmul operand dtype: float8e4 with the DoubleRow perf mode -- both
# 128-contraction halves fold into ONE matmul at 0.5 cyc/output-col (4x the
# bf16/f32r PE rate) and the centers DMA shrinks to 2.1 MB.  Quantization
# noise on the cosine sims is ~3e-3 RMS; end-to-end loss error validated at
# ~6e-4 relative (tolerance 2e-2).  Positives are recomputed exactly on the
# host, and the eps value-matching margins below absorb the noise.
MM_DT = mybir.dt.float8e4
MM_NP = ml_dtypes.float8_e4m3


def _pair_order(sizes):
    """Order cameras big+small so most 128-row tiles span only ~2 cameras."""
    desc = np.argsort(-np.asarray(sizes), kind="stable")
    big, small = desc[: C // 2], desc[C // 2 :][::-1]
    order = []
    for b, s in zip(big, small):
        order += [int(b), int(s)]
    return order


def _unit_plan(tile_cams):
    """Static drain plan: units in b-major order, kind per unit, tree pairs.

    Returns (units, kind, pairs, pair_of, wm_slot): pairs maps a pair id to
    its member units (1 or 2, same row tile); pair_of gives each tree unit
    its (pair id, member slot); wm_slot gives each tree unit its slot in the
    row tile's wm output (pair-order, so tree last rounds are contiguous).
    """
    units = [(rt, b) for b in range(C) for rt in range(RT)]
    kind = {}
    # distribute DIRECT_TOTAL direct units across row tiles (more directs to
    # tiles with fewer exp units)
    nexp = [len(tile_cams[rt]) for rt in range(RT)]
    d_rt = [0] * RT
    for _ in range(DIRECT_TOTAL):
        # give the next direct to the tile with most unassigned non-exp blocks
        loads = [(C - nexp[rt] - d_rt[rt], -rt) for rt in range(RT)]
        rt = -max(loads)[1]
        d_rt[rt] += 1
    # place the direct units so every b-column of the (b-major) drain order
    # gets ~one DVE-drained unit (rotating the row tile): ACT and DVE then
    # stay concurrently busy instead of alternating in bursts
    quota = list(d_rt)
    directs = set()
    for b in range(C):
        cands = [rt for rt in range(RT)
                 if b not in tile_cams[rt] and quota[rt] > 0]
        if not cands:
            continue
        rt = sorted(cands, key=lambda r: (-quota[r], (r - b) % RT))[0]
        directs.add((rt, b))
        quota[rt] -= 1
    for rt in range(RT):
        while quota[rt] > 0:
            ncol = {b: sum(1 for (r2, b2) in directs if b2 == b)
                    for b in range(C)}
            free = [b for b in range(C)
                    if b not in tile_cams[rt] and (rt, b) not in directs]
            b = sorted(free, key=lambda b2: (ncol[b2], b2))[0]
            directs.add((rt, b))
            quota[rt] -= 1
    for rt in range(RT):
        for b in range(C):
            if b in tile_cams[rt]:
                kind[(rt, b)] = "exp"
            elif (rt, b) in directs:
                kind[(rt, b)] = "direct"
            else:
                kind[(rt, b)] = "win"
    # pair tree units (exp+win) within each row tile in b order; a tree
    # unit's wm_rt slot is its position in that order, so every pair's last
    # round writes one contiguous wm_rt slice regardless of block adjacency
    pair_of = {}
    pairs = []
    wm_slot = {}
    for rt in range(RT):
        tus = [(rt, b) for b in range(C) if kind[(rt, b)] != "direct"]
        for s, u in enumerate(tus):
            wm_slot[u] = s
        for i in range(0, len(tus), 2):
            members = tus[i : i + 2]
            pid = len(pairs)
            pairs.append(members)
            for s, u in enumerate(members):
                pair_of[u] = (pid, s)
    return units, kind, pairs, pair_of, wm_slot


@functools.lru_cache(maxsize=8)
def _build_program(tile_cams, repeats=1):
    nc = bacc.Bacc(None, target_bir_lowering=False, num_swdge_queues=4)

    cenTd = nc.dram_tensor("cenT", [2, 128, P_PAD], MM_DT, kind="ExternalInput")
    fTd = nc.dram_tensor("fT", [128, RT, 2, 128], MM_DT, kind="ExternalInput")
    candd = nc.dram_tensor("cand", [RT, 128, C * 8], F32, kind="ExternalOutput")
    srowd = nc.dram_tensor("srow", [RT, 128, C], F32, kind="ExternalOutput")
    wmaxd = nc.dram_tensor("wmax", [RT, 128, C, NW], BF16, kind="ExternalOutput")

    with TileContext(nc) as tc:
        with (
            tc.tile_pool(name="cen", bufs=2) as cenp,
            tc.tile_pool(name="ftp", bufs=2) as ftp,
            tc.tile_pool(name="scrp", bufs=5) as scrp,
            tc.tile_pool(name="treep", bufs=3) as treep,
            tc.tile_pool(name="outp", bufs=2) as outp,
            tc.tile_pool(name="psum", bufs=4, space="PSUM") as psump,
        ):
            for _rep in range(repeats):
                _kernel_body(nc, tc, cenp, ftp, scrp, treep, outp, psump,
                             cenTd, fTd, candd, srowd, wmaxd, tile_cams)

    nc.compile()
    return nc


def _kernel_body(nc, tc, cenp, ftp, scrp, treep, outp, psump,
                 cenTd, fTd, candd, srowd, wmaxd, tile_cams):
    ActF = mybir.ActivationFunctionType

    units, kind, pairs, pair_of, wm_slot = _unit_plan(tile_cams)

    # ---- input DMA: fT first (matmuls need it immediately), then centers in
    # (h, block) granularity so early units unblock fast; alternate issuing
    # engines to spread descriptor generation across queues
    # inputs on SP only (so the next repeat's loads never queue behind this
    # repeat's output descriptors); outputs on gpsimd only
    fT_sb = ftp.tile([128, RT, 2, 128], MM_DT, name="fT_sb")
    nc.sync.dma_start(out=fT_sb[:, :, :, :], in_=fTd[:, :, :, :])
    cen_sb = cenp.tile([128, 2, P_PAD], MM_DT, name="cen_sb")
    for b in range(C):
        sl = slice(b * BPAD, (b + 1) * BPAD)
        for h in range(2):
            nc.sync.dma_start(out=cen_sb[:, h, sl], in_=cenTd[h, :, sl])

    cand_sb = [outp.tile([128, C * 8], F32, name=f"cand{rt}", bufs=2)
               for rt in range(RT)]
    s_t = [outp.tile([128, C], F32, name=f"st{rt}", bufs=2)
           for rt in range(RT)]
    wm_rt = [outp.tile([128, C, NW], BF16, name=f"wm{rt}", bufs=2)
             for rt in range(RT)]

    # pair state: scr tiles allocated lazily, members drain at different times
    pair_scr = [None] * len(pairs)
    pair_filled = [0] * len(pairs)
    # how many direct/exp/tree units remain per rt (to time the output DMAs)
    left_direct = [sum(1 for b in range(C) if kind[(rt, b)] == "direct")
                   for rt in range(RT)]
    left_exp = [len(tile_cams[rt]) for rt in range(RT)]
    left_tree = [sum(1 for b in range(C) if kind[(rt, b)] != "direct")
                 for rt in range(RT)]
    left_slots = list(left_tree)  # used wm slots per rt (ship only those)

    for ui, (rt, b) in enumerate(units):
        ps = psump.tile([128, BPAD], F32, name="ps")
        c0 = b * BPAD
        for j in range(2):
            nc.tensor.matmul(
                ps[:, j * 512 : (j + 1) * 512],
                fT_sb[:, rt, :, :],
                cen_sb[:, :, c0 + j * 512 : c0 + (j + 1) * 512],
                start=True, stop=True,
                perf_mode=mybir.MatmulPerfMode.DoubleRow,
            )
        k = kind[(rt, b)]
        if k == "direct":
            nc.vector.max(cand_sb[rt][:, b * 8 : b * 8 + 8], ps[:, 0:L_LOCAL])
            left_direct[rt] -= 1
            if left_direct[rt] == 0:
                nc.gpsimd.dma_start(out=candd[rt], in_=cand_sb[rt][:, :])
            continue
        pid, slot = pair_of[(rt, b)]
        npair = len(pairs[pid])
        if pair_scr[pid] is None:
            pair_scr[pid] = scrp.tile([128, npair, L_LOCAL], BF16, name="scr")
        scr = pair_scr[pid]
        if k == "exp":
            idx = tile_cams[rt].index(b)
            nc.scalar.activation(
                scr[:, slot, :], ps[:, 0:L_LOCAL], ActF.Exp,
                scale=INV_T, accum_out=s_t[rt][:, idx : idx + 1],
            )
            left_exp[rt] -= 1
            if left_exp[rt] == 0:
                nc.gpsimd.dma_start(out=srowd[rt], in_=s_t[rt][:, :])
        else:
            nc.scalar.copy(scr[:, slot, :], ps[:, 0:L_LOCAL])
        pair_filled[pid] += 1
        if pair_filled[pid] == npair:
            # single-round bf16 max over stride-500 comb windows (window w
            # holds local labels {w, 500+w}): both operands stay packed, so
            # the whole reduction is ONE DVE 2x instruction per pair
            v = scr.rearrange("p np (w nw) -> p np w nw", nw=NW)
            s0 = wm_slot[pairs[pid][0]]
            nc.vector.tensor_max(wm_rt[rt][:, s0 : s0 + npair, :],
                                 v[:, :, 0, :], v[:, :, 1, :])
            left_tree[rt] -= npair
            if left_tree[rt] == 0:
                ntree = left_slots[rt]
                nc.gpsimd.dma_start(out=wmaxd[rt][:, 0:ntree, :],
                                    in_=wm_rt[rt][:, 0:ntree, :])


class _Runner:
    """Sharded 8-core executor for a built Bass program.

    Builds the jax.jit(shard_map(bass_exec)) executable once (the walrus/NEFF
    compile happens inside the first call) and reuses it for every subsequent
    execution, keeping large inputs device-resident.
    """

    def __init__(self, nc, n_cores=NCORES):
        import jax
        from jax.sharding import Mesh, PartitionSpec, NamedSharding
        from jax.experimental.shard_map import shard_map
        from concourse import bass2jax

        self.jax = jax
        self.nc = nc
        self.n_cores = n_cores
        bass2jax.install_neuronx_cc_hook()
        partition_name = (
            nc.partition_id_tensor.name if nc.partition_id_tensor else None
        )
        in_names, out_names, out_avals = [], [], []
        for alloc in nc.m.functions[0].allocations:
            if not isinstance(alloc, mybir.MemoryLocationSet):
                continue
            name = alloc.memorylocations[0].name
            if alloc.kind == "ExternalInput":
                if name != partition_name:
                    in_names.append(name)
            elif alloc.kind == "ExternalOutput":
                out_names.append(name)
                out_avals.append(
                    jax.core.ShapedArray(
                        tuple(alloc.tensor_shape), mybir.dt.np(alloc.dtype)
                    )
                )
        self.in_names, self.out_names, self.out_avals = in_names, out_names, out_avals
        n_params, n_outs = len(in_names), len(out_avals)
        all_in_names = list(in_names) + list(out_names)
        if partition_name is not None:
            all_in_names.append(partition_name)

        def _body(*args):
            operands = list(args)
            if partition_name is not None:
                operands.append(bass2jax.partition_id_tensor())
            return tuple(
                bass2jax._bass_exec_p.bind(
                    *operands,
                    out_avals=tuple(out_avals),
                    in_names=tuple(all_in_names),
                    out_names=tuple(out_names),
                    lowering_input_output_aliases=(),
                    sim_require_finite=True,
                    sim_require_nnan=True,
                    nc=nc,
                )
            )

        devices = jax.devices()[:n_cores]
        self.mesh = Mesh(np.asarray(devices), ("core",))
        self.sh = NamedSharding(self.mesh, PartitionSpec("core"))
        self.fn = jax.jit(
            shard_map(
                _body,
                mesh=self.mesh,
                in_specs=(PartitionSpec("core"),) * (n_params + n_outs),
                out_specs=(PartitionSpec("core"),) * n_outs,
                check_rep=False,
            ),
            donate_argnums=tuple(range(n_params, n_params + n_outs)),
            keep_unused=True,
        )
        self._zero_shapes = [
            ((n_cores * a.shape[0], *a.shape[1:]), a.dtype) for a in out_avals
        ]

    def put_inputs(self, in_maps):
        self.dev_in = [
            self.jax.device_put(
                np.concatenate([np.asarray(m[name]) for m in in_maps], axis=0),
                self.sh,
            )
            for name in self.in_names
        ]

    def _zeros(self):
        return [
            self.jax.device_put(np.zeros(s, d), self.sh)
            for s, d in self._zero_shapes
        ]

    def execute(self):
        outs = self.fn(*self.dev_in, *self._zeros())
        self.jax.block_until_ready(outs)
        return self.unpack(outs)

    def unpack(self, outs):
        return [
            {
                name: np.asarray(outs[i]).reshape(
                    self.n_cores, *self.out_avals[i].shape
                )[c]
                for i, name in enumerate(self.out_names)
            }
            for c in range(self.n_cores)
        ]


_RUNNERS = {}
_LAST_FALLBACKS = 0
_FORCE_FALLBACK = False  # test hook: exercise the exact host fallback path


def _get_runner(nc):
    r = _RUNNERS.get(id(nc))
    if r is None:
        r = _Runner(nc)
        _RUNNERS[id(nc)] = r
    return r


def _make_in_maps(cenT_shards, feats_p):
    # feats_p is the permuted, L2-normalized batch; device wants the
    # transposed layout [q, rt, h, r] with q the contraction partition
    fT = np.ascontiguousarray(
        feats_p.reshape(RT, 128, 2, 128).transpose(3, 0, 2, 1), dtype=MM_NP
    )
    return [
        {"cenT": np.ascontiguousarray(cenT_shards[k], dtype=MM_NP), "fT": fT}
        for k in range(NCORES)
    ]


def _host_finish(results, feats_p, labels_p, cams_p, centers, tile_cams):
    units, kind, pairs, pair_of, wm_slot = _unit_plan(tile_cams)
    rows = np.arange(N)
    rt_of = rows // 128
    p_of = rows % 128

    # ---- intra: sum over cores of per-camera exp sums ----
    slot = np.zeros(N, dtype=np.int64)
    for rt in range(RT):
        for idx, cam in enumerate(tile_cams[rt]):
            sel = slice(128 * rt, 128 * (rt + 1))
            slot[sel] = np.where(cams_p[sel] == cam, idx, slot[sel])
    s_k = np.stack(
        [
            results[k]["srow"].reshape(RT, 128, C)[rt_of, p_of, slot]
            for k in range(NCORES)
        ]
    ).astype(np.float64)  # [8, 512]: sum_l exp(20 * cos sims) per core

    fn = feats_p.astype(np.float64)
    fn = fn / np.linalg.norm(fn, axis=1, keepdims=True)
    cen = centers.astype(np.float64)
    gidx = labels_p[:, None] * C + np.arange(C)[None, :]        # [512, 8]
    pos = np.einsum("rcd,rd->rc", cen[gidx], fn)                # [512, 8] f64

    lse_intra = np.log(s_k.sum(axis=0))
    v = pos[rows, cams_p]
    loss_intra_i = lse_intra - INV_T * v

    # ---- inter: merge candidates ----
    # direct blocks contribute their top-8 values; tree blocks their 125
    # window maxes (exp blocks in exp domain: s = log(w)/20).
    cand = np.stack([results[k]["cand"] for k in range(NCORES)])  # [8,RT,128,64]
    wmraw = np.stack(
        [results[k]["wmax"].astype(np.float32) for k in range(NCORES)]
    )  # [8,RT,128,C,NW], slot-indexed per row tile

    # remap slots -> blocks, converting exp-domain window maxes back to sims
    # domain; direct blocks have no window data (-inf)
    wmax = np.full_like(wmraw, -np.inf)
    for rt in range(RT):
        for b in range(C):
            kd = kind[(rt, b)]
            if kd == "direct":
                continue
            w = wmraw[:, rt, :, wm_slot[(rt, b)], :]
            if kd == "exp":
                w = np.log(np.maximum(w, 1e-30)) / INV_T
            wmax[:, rt, :, b, :] = w

    wspan = C * NW
    cspan = C * 8
    span = wspan + cspan
    CR = np.empty((N, NCORES * span), dtype=np.float64)
    # window part: CR[i, k*span + b*NW + w]
    CR[:, : NCORES * wspan].reshape(N, NCORES, wspan)[:] = (
        wmax[:, rt_of, p_of, :, :].transpose(1, 0, 2, 3).reshape(N, NCORES, wspan)
    )
    # direct part: CR[i, NCORES*wspan + k*cspan + b*8 + j], -inf for non-direct
    cpart = cand[:, rt_of, p_of, :].transpose(1, 0, 2)          # [512, 8, 64]
    dmask = np.zeros((N, cspan), dtype=bool)
    for rt in range(RT):
        rsel = rt_of == rt
        for b in range(C):
            if kind[(rt, b)] == "direct":
                dmask[rsel, b * 8 : b * 8 + 8] = True
    CR[:, NCORES * wspan :] = np.where(
        dmask[:, None, :], cpart, -np.inf
    ).reshape(N, NCORES * cspan)

    # ---- remove positives by eps value matching ----
    owner = labels_p // L_LOCAL
    lloc = labels_p % L_LOCAL
    win = lloc % NW  # stride-125 comb windows
    EPS = 1.5e-2
    for i in rows:
        rt = rt_of[i]
        k0 = owner[i]
        for c in range(C):
            if kind[(rt, c)] == "direct":
                idxs = np.arange(NCORES * wspan + k0 * cspan + c * 8,
                                 NCORES * wspan + k0 * cspan + c * 8 + 8)
                vals = CR[i, idxs]
                j = int(np.argmin(np.abs(vals - pos[i, c])))
                if abs(vals[j] - pos[i, c]) < EPS:
                    CR[i, idxs[j]] = -np.inf
            else:
                jj = k0 * wspan + c * NW + win[i]
                if abs(CR[i, jj] - pos[i, c]) < EPS:
                    CR[i, jj] = -np.inf

    part = np.partition(CR, CR.shape[1] - K, axis=1)[:, -K:]
    t50 = part.min(axis=1)

    # ---- certificate on direct blocks: 8th value must be <= t50 ----
    if _FORCE_FALLBACK:
        bad = rows
    else:
        worst = np.full(N, -np.inf)
        for rt in range(RT):
            rsel = rt_of == rt
            for b in range(C):
                if kind[(rt, b)] != "direct":
                    continue
                worst[rsel] = np.maximum(
                    worst[rsel],
                    cand[:, rt, :, b * 8 + 7].max(axis=0)[p_of[rsel]],
                )
        bad = np.where(worst > t50)[0]
    global _LAST_FALLBACKS
    _LAST_FALLBACKS = len(bad)
    for i in bad:
        sims_row = cen @ fn[i]                                  # [64000] exact
        sims_row[C * labels_p[i] : C * labels_p[i] + C] = -np.inf
        part[i] = np.sort(sims_row)[-K:]

    z = np.concatenate([pos, part], axis=1) * INV_T             # [512, 58]
    mz = z.max(axis=1)
    lse_inter = np.log(np.exp(z - mz[:, None]).sum(axis=1)) + mz
    loss_inter_i = lse_inter - INV_T * pos.mean(axis=1)

    # ---- per-camera means, summed ----
    cnt = np.bincount(cams_p, minlength=C).astype(np.float64)
    s_intra = np.bincount(cams_p, weights=loss_intra_i, minlength=C)
    s_inter = np.bincount(cams_p, weights=loss_inter_i, minlength=C)
    safe = np.maximum(cnt, 1.0)
    li = np.sum(np.where(cnt > 0, s_intra / safe, 0.0))
    le = LW * np.sum(np.where(cnt > 0, s_inter / safe, 0.0))
    return np.array([li, le], dtype=np.float32)


def _prepare(feats, indexes, label_table, cam_table, centers):
    feats = np.asarray(feats, dtype=np.float32)
    indexes = np.asarray(indexes)
    label_table = np.asarray(label_table)
    cam_table = np.asarray(cam_table)
    centers = np.asarray(centers, dtype=np.float32)

    labels = np.asarray(label_table[indexes], dtype=np.int64)
    cams = np.asarray(cam_table[indexes], dtype=np.int64)

    # permute rows so camera groups are contiguous, ordered big+small so most
    # 128-row tiles span only ~2 cameras (fewer intra exp instructions)
    sizes = np.bincount(cams, minlength=C)
    order = _pair_order(sizes)
    perm = np.concatenate([np.where(cams == c)[0] for c in order])
    fp = feats[perm].astype(np.float64)
    fp = fp / np.linalg.norm(fp, axis=1, keepdims=True)
    feats_p = np.ascontiguousarray(fp, dtype=np.float32)
    labels_p = labels[perm]
    cams_p = cams[perm]
    tile_cams = tuple(
        tuple(dict.fromkeys(cams_p[128 * rt : 128 * (rt + 1)].tolist()))
        for rt in range(RT)
    )
    # camera-major padded center shards: [2, 128, 8192] per core
    cenT_shards = []
    for k in range(NCORES):
        ck = centers[k * L_LOCAL * C : (k + 1) * L_LOCAL * C]
        ck = ck.reshape(L_LOCAL, C, D).transpose(1, 0, 2)   # [C, 1000, 256]
        pad = np.zeros((C, BPAD - L_LOCAL, D), dtype=np.float32)
        ckp = np.concatenate([ck, pad], axis=1)             # [C, 1024, 256]
        cenT = ckp.reshape(P_PAD, D).T                      # [256, 8192]
        cenT_shards.append(
            np.ascontiguousarray(cenT.reshape(2, 128, P_PAD), dtype=MM_NP)
        )
    return centers, tile_cams, feats_p, labels_p, cams_p, cenT_shards


def kernel(feats, indexes, label_table, cam_table, centers):
    centers, tile_cams, feats_p, labels_p, cams_p, cenT_shards = _prepare(
        feats, indexes, label_table, cam_table, centers
    )
    nc = _build_program(tile_cams)
    runner = _get_runner(nc)
    runner.put_inputs(_make_in_maps(cenT_shards, feats_p))
    results = runner.execute()
    return _host_finish(results, feats_p, labels_p, cams_p, centers, tile_cams)


# revision 51
# speedup vs baseline: 1.2940x; 1.0286x over previous
"""Trainium2 Bass kernel for nn_CAPMemory (camera-aware proxy memory loss).

Strategy (8 NeuronCores, SPMD, no collectives):
  - Shard the 64000x256 proxy table over labels: core k owns labels
    [1000k, 1000(k+1)), all 8 cameras.  On the host the shard is laid out
    CAMERA-MAJOR with each camera block padded 1000 -> 1024 columns
    (pad centers = 0 vectors): col c*1024 + l holds proxy (label l, cam c).
    This aligns camera blocks with PSUM banks and makes the intra-camera
    softmax read contiguous.
  - Feats are normalized and transposed on the host; the device runs a pure
    pipeline: DMA -> fp8e4 DoubleRow matmul (both 128-contraction halves in
    one instruction) -> per-1024-column-unit PSUM drain -> small outputs.
  - Each (row-tile, camera-block) unit [128 x 1024] in PSUM is drained by
    exactly one engine:
      direct unit: DVE MAX8 straight from PSUM -> top-8 values (f32)
      exp unit   : ACT Exp(scale=20) from PSUM -> bf16 exp values in SBUF
                   (+ accumulated per-camera exp-sum for the intra loss)
      window unit: ACT Copy from PSUM -> bf16 sims in SBUF
    bf16 units are then reduced over stride-500 comb windows (window w =
    labels {w, 500+w}) in a SINGLE DVE 2x tensor_max per pair, giving
    2-element window maxes [128 x 500] shipped to the host (exp-domain
    values for exp units; the host takes log/20).
  - Host merge: intra logsumexp = log(sum_k srow_k); inter top-50 hard
    negatives merged from per-block top-8s and window maxes, positives
    removed by eps value matching; positives themselves recomputed exactly
    on host in f64.
  - Certificate on direct blocks (8th value <= merged t50) triggers exact
    per-row host recomputation; window blocks are statistically covered
    (window collisions lose at most one near-cutoff negative, effect on the
    loss ~1e-4 relative, validated offline against the reference).
"""

import sys
import functools

sys.path.insert(0, "/opt/trn_rl_repo")

import numpy as np
import ml_dtypes

from concourse import bacc, mybir
from concourse.tile import TileContext

F32 = mybir.dt.float32
BF16 = mybir.dt.bfloat16

N = 512          # batch
D = 256          # feature dim
L = 8000         # labels
C = 8            # cameras
NCORES = 8
RT = 4           # row tiles of 128
L_LOCAL = 1000   # labels per core
BPAD = 1024      # padded camera-block width
P_PAD = C * BPAD  # 8192 padded columns per core
INV_T = 20.0     # 1 / temperature
K = 50           # hard negatives
LW = 0.5         # inter-cam loss weight
NW = L_LOCAL // 2  # 500 2-wide comb windows per camera block

# total units drained by DVE MAX8 straight from PSUM (top-8 candidates);
# the rest are drained by ACT (exp or copy) and window-max-treed on DVE.
# ACT and DVE are the only engines that can read PSUM, so this splits the
# [512 x 8192] drain across both.
DIRECT_TOTAL = 12

# matmul operand dtype: float8e4 with the DoubleRow perf mode -- both
# 128-contraction halves fold into ONE matmul at 0.5 cyc/output-col (4x the
# bf16/f32r PE rate) and the centers DMA shrinks to 2.1 MB.  Quantization
# noise on the cosine sims is ~3e-3 RMS; end-to-end loss error validated at
# ~6e-4 relative (tolerance 2e-2).  Positives are recomputed exactly on the
# host, and the eps value-matching margins below absorb the noise.
MM_DT = mybir.dt.float8e4
MM_NP = ml_dtypes.float8_e4m3


def _pair_order(sizes):
    """Order cameras so the fewest cameras straddle 128-row tile boundaries
    (each straddle costs one extra intra-exp instruction on ACT)."""
    import itertools
    sz = [int(s) for s in np.asarray(sizes)]

    def exp_count(order):
        edges = np.cumsum([0] + [sz[c] for c in order])
        tot = 0
        for rt in range(RT):
            lo, hi = rt * 128, (rt + 1) * 128
            tot += sum(1 for i in range(C)
                       if edges[i] < hi and edges[i + 1] > lo)
        return tot

    best = min(itertools.permutations(range(C)), key=exp_count)
    return [int(c) for c in best]


def _unit_plan(tile_cams):
    """Static drain plan: units in b-major order, kind per unit, tree pairs.

    Returns (units, kind, pairs, pair_of, wm_slot): pairs maps a pair id to
    its member units (1 or 2, same row tile); pair_of gives each tree unit
    its (pair id, member slot); wm_slot gives each tree unit its slot in the
    row tile's wm output (pair-order, so tree last rounds are contiguous).
    """
    units = [(rt, b) for b in range(C) for rt in range(RT)]
    kind = {}
    # distribute DIRECT_TOTAL direct units across row tiles (more directs to
    # tiles with fewer exp units)
    nexp = [len(tile_cams[rt]) for rt in range(RT)]
    d_rt = [0] * RT
    for _ in range(DIRECT_TOTAL):
        # give the next direct to the tile with most unassigned non-exp blocks
        loads = [(C - nexp[rt] - d_rt[rt], -rt) for rt in range(RT)]
        rt = -max(loads)[1]
        d_rt[rt] += 1
    # place the direct units so every b-column of the (b-major) drain order
    # gets ~one DVE-drained unit (rotating the row tile): ACT and DVE then
    # stay concurrently busy instead of alternating in bursts
    quota = list(d_rt)
    directs = set()
    for b in range(C):
        cands = [rt for rt in range(RT)
                 if b not in tile_cams[rt] and quota[rt] > 0]
        if not cands:
            continue
        rt = sorted(cands, key=lambda r: (-quota[r], (r - b) % RT))[0]
        directs.add((rt, b))
        quota[rt] -= 1
    for rt in range(RT):
        while quota[rt] > 0:
            ncol = {b: sum(1 for (r2, b2) in directs if b2 == b)
                    for b in range(C)}
            free = [b for b in range(C)
                    if b not in tile_cams[rt] and (rt, b) not in directs]
            b = sorted(free, key=lambda b2: (ncol[b2], b2))[0]
            directs.add((rt, b))
            quota[rt] -= 1
    for rt in range(RT):
        for b in range(C):
            if b in tile_cams[rt]:
                kind[(rt, b)] = "exp"
            elif (rt, b) in directs:
                kind[(rt, b)] = "direct"
            else:
                kind[(rt, b)] = "win"
    # pair tree units (exp+win) within each row tile in b order; a tree
    # unit's wm_rt slot is its position in that order, so every pair's last
    # round writes one contiguous wm_rt slice regardless of block adjacency
    pair_of = {}
    pairs = []
    wm_slot = {}
    for rt in range(RT):
        tus = [(rt, b) for b in range(C) if kind[(rt, b)] != "direct"]
        for s, u in enumerate(tus):
            wm_slot[u] = s
        for i in range(0, len(tus), 2):
            members = tus[i : i + 2]
            pid = len(pairs)
            pairs.append(members)
            for s, u in enumerate(members):
                pair_of[u] = (pid, s)
    return units, kind, pairs, pair_of, wm_slot


@functools.lru_cache(maxsize=8)
def _build_program(tile_cams, repeats=1):
    nc = bacc.Bacc(None, target_bir_lowering=False, num_swdge_queues=4)

    cenTd = nc.dram_tensor("cenT", [2, 128, P_PAD], MM_DT, kind="ExternalInput")
    fTd = nc.dram_tensor("fT", [128, RT, 2, 128], MM_DT, kind="ExternalInput")
    candd = nc.dram_tensor("cand", [RT, 128, C * 8], F32, kind="ExternalOutput")
    srowd = nc.dram_tensor("srow", [RT, 128, C], F32, kind="ExternalOutput")
    wmaxd = nc.dram_tensor("wmax", [RT, 128, C, NW], BF16, kind="ExternalOutput")

    with TileContext(nc) as tc:
        with (
            tc.tile_pool(name="cen", bufs=2) as cenp,
            tc.tile_pool(name="ftp", bufs=2) as ftp,
            tc.tile_pool(name="scrp", bufs=5) as scrp,
            tc.tile_pool(name="treep", bufs=3) as treep,
            tc.tile_pool(name="outp", bufs=2) as outp,
            tc.tile_pool(name="psum", bufs=4, space="PSUM") as psump,
        ):
            for _rep in range(repeats):
                _kernel_body(nc, tc, cenp, ftp, scrp, treep, outp, psump,
                             cenTd, fTd, candd, srowd, wmaxd, tile_cams)

    nc.compile()
    return nc


def _kernel_body(nc, tc, cenp, ftp, scrp, treep, outp, psump,
                 cenTd, fTd, candd, srowd, wmaxd, tile_cams):
    ActF = mybir.ActivationFunctionType

    units, kind, pairs, pair_of, wm_slot = _unit_plan(tile_cams)

    # ---- input DMA: fT first (matmuls need it immediately), then centers in
    # (h, block) granularity so early units unblock fast; alternate issuing
    # engines to spread descriptor generation across queues
    # inputs on SP only (so the next repeat's loads never queue behind this
    # repeat's output descriptors); outputs on gpsimd only
    fT_sb = ftp.tile([128, RT, 2, 128], MM_DT, name="fT_sb")
    nc.sync.dma_start(out=fT_sb[:, :, :, :], in_=fTd[:, :, :, :])
    cen_sb = cenp.tile([128, 2, P_PAD], MM_DT, name="cen_sb")
    for b in range(C):
        sl = slice(b * BPAD, (b + 1) * BPAD)
        for h in range(2):
            nc.sync.dma_start(out=cen_sb[:, h, sl], in_=cenTd[h, :, sl])

    cand_sb = [outp.tile([128, C * 8], F32, name=f"cand{rt}", bufs=2)
               for rt in range(RT)]
    s_t = [outp.tile([128, C], F32, name=f"st{rt}", bufs=2)
           for rt in range(RT)]
    wm_rt = [outp.tile([128, C, NW], BF16, name=f"wm{rt}", bufs=2)
             for rt in range(RT)]

    # pair state: scr tiles allocated lazily, members drain at different times
    pair_scr = [None] * len(pairs)
    pair_filled = [0] * len(pairs)
    # how many direct/exp/tree units remain per rt (to time the output DMAs)
    left_direct = [sum(1 for b in range(C) if kind[(rt, b)] == "direct")
                   for rt in range(RT)]
    left_exp = [len(tile_cams[rt]) for rt in range(RT)]
    left_tree = [sum(1 for b in range(C) if kind[(rt, b)] != "direct")
                 for rt in range(RT)]
    left_slots = list(left_tree)  # used wm slots per rt (ship only those)

    for ui, (rt, b) in enumerate(units):
        ps = psump.tile([128, BPAD], F32, name="ps")
        c0 = b * BPAD
        for j in range(2):
            nc.tensor.matmul(
                ps[:, j * 512 : (j + 1) * 512],
                fT_sb[:, rt, :, :],
                cen_sb[:, :, c0 + j * 512 : c0 + (j + 1) * 512],
                start=True, stop=True,
                perf_mode=mybir.MatmulPerfMode.DoubleRow,
            )
        k = kind[(rt, b)]
        if k == "direct":
            nc.vector.max(cand_sb[rt][:, b * 8 : b * 8 + 8], ps[:, 0:L_LOCAL])
            left_direct[rt] -= 1
            if left_direct[rt] == 0:
                nc.gpsimd.dma_start(out=candd[rt], in_=cand_sb[rt][:, :])
            continue
        pid, slot = pair_of[(rt, b)]
        npair = len(pairs[pid])
        if pair_scr[pid] is None:
            pair_scr[pid] = scrp.tile([128, npair, L_LOCAL], BF16, name="scr")
        scr = pair_scr[pid]
        if k == "exp":
            idx = tile_cams[rt].index(b)
            nc.scalar.activation(
                scr[:, slot, :], ps[:, 0:L_LOCAL], ActF.Exp,
                scale=INV_T, accum_out=s_t[rt][:, idx : idx + 1],
            )
            left_exp[rt] -= 1
            if left_exp[rt] == 0:
                nc.gpsimd.dma_start(out=srowd[rt], in_=s_t[rt][:, :])
        else:
            nc.scalar.copy(scr[:, slot, :], ps[:, 0:L_LOCAL])
        pair_filled[pid] += 1
        if pair_filled[pid] == npair:
            # single-round bf16 max over stride-500 comb windows (window w
            # holds local labels {w, 500+w}): both operands stay packed, so
            # the whole reduction is ONE DVE 2x instruction per pair
            v = scr.rearrange("p np (w nw) -> p np w nw", nw=NW)
            s0 = wm_slot[pairs[pid][0]]
            nc.vector.tensor_max(wm_rt[rt][:, s0 : s0 + npair, :],
                                 v[:, :, 0, :], v[:, :, 1, :])
            left_tree[rt] -= npair
            if left_tree[rt] == 0:
                ntree = left_slots[rt]
                nc.gpsimd.dma_start(out=wmaxd[rt][:, 0:ntree, :],
                                    in_=wm_rt[rt][:, 0:ntree, :])


class _Runner:
    """Sharded 8-core executor for a built Bass program.

    Builds the jax.jit(shard_map(bass_exec)) executable once (the walrus/NEFF
    compile happens inside the first call) and reuses it for every subsequent
    execution, keeping large inputs device-resident.
    """

    def __init__(self, nc, n_cores=NCORES):
        import jax
        from jax.sharding import Mesh, PartitionSpec, NamedSharding
        from jax.experimental.shard_map import shard_map
        from concourse import bass2jax

        self.jax = jax
        self.nc = nc
        self.n_cores = n_cores
        bass2jax.install_neuronx_cc_hook()
        partition_name = (
            nc.partition_id_tensor.name if nc.partition_id_tensor else None
        )
        in_names, out_names, out_avals = [], [], []
        for alloc in nc.m.functions[0].allocations:
            if not isinstance(alloc, mybir.MemoryLocationSet):
                continue
            name = alloc.memorylocations[0].name
            if alloc.kind == "ExternalInput":
                if name != partition_name:
                    in_names.append(name)
            elif alloc.kind == "ExternalOutput":
                out_names.append(name)
                out_avals.append(
                    jax.core.ShapedArray(
                        tuple(alloc.tensor_shape), mybir.dt.np(alloc.dtype)
                    )
                )
        self.in_names, self.out_names, self.out_avals = in_names, out_names, out_avals
        n_params, n_outs = len(in_names), len(out_avals)
        all_in_names = list(in_names) + list(out_names)
        if partition_name is not None:
            all_in_names.append(partition_name)

        def _body(*args):
            operands = list(args)
            if partition_name is not None:
                operands.append(bass2jax.partition_id_tensor())
            return tuple(
                bass2jax._bass_exec_p.bind(
                    *operands,
                    out_avals=tuple(out_avals),
                    in_names=tuple(all_in_names),
                    out_names=tuple(out_names),
                    lowering_input_output_aliases=(),
                    sim_require_finite=True,
                    sim_require_nnan=True,
                    nc=nc,
                )
            )

        devices = jax.devices()[:n_cores]
        self.mesh = Mesh(np.asarray(devices), ("core",))
        self.sh = NamedSharding(self.mesh, PartitionSpec("core"))
        self.fn = jax.jit(
            shard_map(
                _body,
                mesh=self.mesh,
                in_specs=(PartitionSpec("core"),) * (n_params + n_outs),
                out_specs=(PartitionSpec("core"),) * n_outs,
                check_rep=False,
            ),
            donate_argnums=tuple(range(n_params, n_params + n_outs)),
            keep_unused=True,
        )
        self._zero_shapes = [
            ((n_cores * a.shape[0], *a.shape[1:]), a.dtype) for a in out_avals
        ]

    def put_inputs(self, in_maps):
        self.dev_in = [
            self.jax.device_put(
                np.concatenate([np.asarray(m[name]) for m in in_maps], axis=0),
                self.sh,
            )
            for name in self.in_names
        ]

    def _zeros(self):
        return [
            self.jax.device_put(np.zeros(s, d), self.sh)
            for s, d in self._zero_shapes
        ]

    def execute(self):
        outs = self.fn(*self.dev_in, *self._zeros())
        self.jax.block_until_ready(outs)
        return self.unpack(outs)

    def unpack(self, outs):
        return [
            {
                name: np.asarray(outs[i]).reshape(
                    self.n_cores, *self.out_avals[i].shape
                )[c]
                for i, name in enumerate(self.out_names)
            }
            for c in range(self.n_cores)
        ]


_RUNNERS = {}
_LAST_FALLBACKS = 0
_FORCE_FALLBACK = False  # test hook: exercise the exact host fallback path


def _get_runner(nc):
    r = _RUNNERS.get(id(nc))
    if r is None:
        r = _Runner(nc)
        _RUNNERS[id(nc)] = r
    return r


def _make_in_maps(cenT_shards, feats_p):
    # feats_p is the permuted, L2-normalized batch; device wants the
    # transposed layout [q, rt, h, r] with q the contraction partition
    fT = np.ascontiguousarray(
        feats_p.reshape(RT, 128, 2, 128).transpose(3, 0, 2, 1), dtype=MM_NP
    )
    return [
        {"cenT": np.ascontiguousarray(cenT_shards[k], dtype=MM_NP), "fT": fT}
        for k in range(NCORES)
    ]


def _host_finish(results, feats_p, labels_p, cams_p, centers, tile_cams):
    units, kind, pairs, pair_of, wm_slot = _unit_plan(tile_cams)
    rows = np.arange(N)
    rt_of = rows // 128
    p_of = rows % 128

    # ---- intra: sum over cores of per-camera exp sums ----
    slot = np.zeros(N, dtype=np.int64)
    for rt in range(RT):
        for idx, cam in enumerate(tile_cams[rt]):
            sel = slice(128 * rt, 128 * (rt + 1))
            slot[sel] = np.where(cams_p[sel] == cam, idx, slot[sel])
    s_k = np.stack(
        [
            results[k]["srow"].reshape(RT, 128, C)[rt_of, p_of, slot]
            for k in range(NCORES)
        ]
    ).astype(np.float64)  # [8, 512]: sum_l exp(20 * cos sims) per core

    fn = feats_p.astype(np.float64)
    fn = fn / np.linalg.norm(fn, axis=1, keepdims=True)
    cen = centers.astype(np.float64)
    gidx = labels_p[:, None] * C + np.arange(C)[None, :]        # [512, 8]
    pos = np.einsum("rcd,rd->rc", cen[gidx], fn)                # [512, 8] f64

    lse_intra = np.log(s_k.sum(axis=0))
    v = pos[rows, cams_p]
    loss_intra_i = lse_intra - INV_T * v

    # ---- inter: merge candidates ----
    # direct blocks contribute their top-8 values; tree blocks their 125
    # window maxes (exp blocks in exp domain: s = log(w)/20).
    cand = np.stack([results[k]["cand"] for k in range(NCORES)])  # [8,RT,128,64]
    wmraw = np.stack(
        [results[k]["wmax"].astype(np.float32) for k in range(NCORES)]
    )  # [8,RT,128,C,NW], slot-indexed per row tile

    # remap slots -> blocks, converting exp-domain window maxes back to sims
    # domain; direct blocks have no window data (-inf)
    wmax = np.full_like(wmraw, -np.inf)
    for rt in range(RT):
        for b in range(C):
            kd = kind[(rt, b)]
            if kd == "direct":
                continue
            w = wmraw[:, rt, :, wm_slot[(rt, b)], :]
            if kd == "exp":
                w = np.log(np.maximum(w, 1e-30)) / INV_T
            wmax[:, rt, :, b, :] = w

    wspan = C * NW
    cspan = C * 8
    span = wspan + cspan
    CR = np.empty((N, NCORES * span), dtype=np.float64)
    # window part: CR[i, k*span + b*NW + w]
    CR[:, : NCORES * wspan].reshape(N, NCORES, wspan)[:] = (
        wmax[:, rt_of, p_of, :, :].transpose(1, 0, 2, 3).reshape(N, NCORES, wspan)
    )
    # direct part: CR[i, NCORES*wspan + k*cspan + b*8 + j], -inf for non-direct
    cpart = cand[:, rt_of, p_of, :].transpose(1, 0, 2)          # [512, 8, 64]
    dmask = np.zeros((N, cspan), dtype=bool)
    for rt in range(RT):
        rsel = rt_of == rt
        for b in range(C):
            if kind[(rt, b)] == "direct":
                dmask[rsel, b * 8 : b * 8 + 8] = True
    CR[:, NCORES * wspan :] = np.where(
        dmask[:, None, :], cpart, -np.inf
    ).reshape(N, NCORES * cspan)

    # ---- remove positives by eps value matching ----
    owner = labels_p // L_LOCAL
    lloc = labels_p % L_LOCAL
    win = lloc % NW  # stride-125 comb windows
    EPS = 1.5e-2
    for i in rows:
        rt = rt_of[i]
        k0 = owner[i]
        for c in range(C):
            if kind[(rt, c)] == "direct":
                idxs = np.arange(NCORES * wspan + k0 * cspan + c * 8,
                                 NCORES * wspan + k0 * cspan + c * 8 + 8)
                vals = CR[i, idxs]
                j = int(np.argmin(np.abs(vals - pos[i, c])))
                if abs(vals[j] - pos[i, c]) < EPS:
                    CR[i, idxs[j]] = -np.inf
            else:
                jj = k0 * wspan + c * NW + win[i]
                if abs(CR[i, jj] - pos[i, c]) < EPS:
                    CR[i, jj] = -np.inf

    part = np.partition(CR, CR.shape[1] - K, axis=1)[:, -K:]
    t50 = part.min(axis=1)

    # ---- certificate on direct blocks: 8th value must be <= t50 ----
    if _FORCE_FALLBACK:
        bad = rows
    else:
        worst = np.full(N, -np.inf)
        for rt in range(RT):
            rsel = rt_of == rt
            for b in range(C):
                if kind[(rt, b)] != "direct":
                    continue
                worst[rsel] = np.maximum(
                    worst[rsel],
                    cand[:, rt, :, b * 8 + 7].max(axis=0)[p_of[rsel]],
                )
        bad = np.where(worst > t50)[0]
    global _LAST_FALLBACKS
    _LAST_FALLBACKS = len(bad)
    for i in bad:
        sims_row = cen @ fn[i]                                  # [64000] exact
        sims_row[C * labels_p[i] : C * labels_p[i] + C] = -np.inf
        part[i] = np.sort(sims_row)[-K:]

    z = np.concatenate([pos, part], axis=1) * INV_T             # [512, 58]
    mz = z.max(axis=1)
    lse_inter = np.log(np.exp(z - mz[:, None]).sum(axis=1)) + mz
    loss_inter_i = lse_inter - INV_T * pos.mean(axis=1)

    # ---- per-camera means, summed ----
    cnt = np.bincount(cams_p, minlength=C).astype(np.float64)
    s_intra = np.bincount(cams_p, weights=loss_intra_i, minlength=C)
    s_inter = np.bincount(cams_p, weights=loss_inter_i, minlength=C)
    safe = np.maximum(cnt, 1.0)
    li = np.sum(np.where(cnt > 0, s_intra / safe, 0.0))
    le = LW * np.sum(np.where(cnt > 0, s_inter / safe, 0.0))
    return np.array([li, le], dtype=np.float32)


def _prepare(feats, indexes, label_table, cam_table, centers):
    feats = np.asarray(feats, dtype=np.float32)
    indexes = np.asarray(indexes)
    label_table = np.asarray(label_table)
    cam_table = np.asarray(cam_table)
    centers = np.asarray(centers, dtype=np.float32)

    labels = np.asarray(label_table[indexes], dtype=np.int64)
    cams = np.asarray(cam_table[indexes], dtype=np.int64)

    # permute rows so camera groups are contiguous, ordered big+small so most
    # 128-row tiles span only ~2 cameras (fewer intra exp instructions)
    sizes = np.bincount(cams, minlength=C)
    order = _pair_order(sizes)
    perm = np.concatenate([np.where(cams == c)[0] for c in order])
    fp = feats[perm].astype(np.float64)
    fp = fp / np.linalg.norm(fp, axis=1, keepdims=True)
    feats_p = np.ascontiguousarray(fp, dtype=np.float32)
    labels_p = labels[perm]
    cams_p = cams[perm]
    tile_cams = tuple(
        tuple(dict.fromkeys(cams_p[128 * rt : 128 * (rt + 1)].tolist()))
        for rt in range(RT)
    )
    # camera-major padded center shards: [2, 128, 8192] per core
    cenT_shards = []
    for k in range(NCORES):
        ck = centers[k * L_LOCAL * C : (k + 1) * L_LOCAL * C]
        ck = ck.reshape(L_LOCAL, C, D).transpose(1, 0, 2)   # [C, 1000, 256]
        pad = np.zeros((C, BPAD - L_LOCAL, D), dtype=np.float32)
        ckp = np.concatenate([ck, pad], axis=1)             # [C, 1024, 256]
        cenT = ckp.reshape(P_PAD, D).T                      # [256, 8192]
        cenT_shards.append(
            np.ascontiguousarray(cenT.reshape(2, 128, P_PAD), dtype=MM_NP)
        )
    return centers, tile_cams, feats_p, labels_p, cams_p, cenT_shards


def kernel(feats, indexes, label_table, cam_table, centers):
    centers, tile_cams, feats_p, labels_p, cams_p, cenT_shards = _prepare(
        feats, indexes, label_table, cam_table, centers
    )
    nc = _build_program(tile_cams)
    runner = _get_runner(nc)
    runner.put_inputs(_make_in_maps(cenT_shards, feats_p))
    results = runner.execute()
    return _host_finish(results, feats_p, labels_p, cams_p, centers, tile_cams)
